# revision 1
# baseline (speedup 1.0000x reference)
"""Trainium2 Bass kernel for nn_Attention_12463995093474 (sparse_attention).

Math (reference):
  q/k/v = content linears; 2 absolute heads, 2 relative heads (DK=32).
  abs:  scores = (Xq_a + abs_kernel@abs_q_w) @ (Xk_a + abs_kernel@abs_k_w)^T
  rel:  scores = Xq_r @ Xk_r^T + (Xq_r + rel_bias) . (rel_kernel@rel_k_w + rel_k_b)
  softmax(mask) @ v -> out linear.

The dominant term is the streaming contraction over rel_kernel (655 MB):
    s2[i,j] = sum_o q'[i,o] * (sum_d RK[i,j,d] rel_k_w[d,o]) (+ const c[i])
This version runs it almost entirely on the TensorEngine:
  1. dma_start_transpose streams RK[i, j, :] chunks HBM->SBUF as [d=128, (i,j)]
     (bf16 xbar transpose).
  2. Stage B (PE): for groups of 4 i's, 4 col-tiled matmuls with
     lhsT = rel_k_w [128d, 32o] write R^T packed as [4i x 32o, 200j] in PSUM.
  3. A PSUM->SBUF bf16 copy (DVE/ACT alternating).
  4. Stage C (PE): per 32-i band, 8 accumulating matmuls with a
     zero-interleaved block-diagonal lhsT holding q' values contract o away,
     producing s2[i-band, j] directly in PSUM (32-aligned output bands).
  5. The content-score matmul for rel heads accumulates into the same PSUM
     tile (start=False), so the epilogue reads finished scores.
This removes the 41M-element DVE multiply+tree-reduce of the previous
version; the kernel becomes transpose-DMA bound instead of DVE bound.

Sharding: data-parallel over batch, B=16 -> 2 batches per core on 8 cores.
"""

import numpy as np
from contextlib import ExitStack

import concourse.bass as bass
import concourse.bacc as bacc
import concourse.tile as tile
from concourse import mybir
from concourse.masks import make_identity
from concourse.bass_utils import run_bass_kernel_spmd

B, T, D = 16, 200, 128
H_ABS, H_REL, H, DK = 2, 2, 4, 32
N_CORES = 8
BPC = B // N_CORES  # batches per core
SCALE = 1.0 / float(DK) ** 0.5
TT = BPC * T  # tokens per core (400)

F32 = mybir.dt.float32
BF16 = mybir.dt.bfloat16
F8 = mybir.dt.float8e4
I32 = mybir.dt.int32
AX = mybir.AxisListType
OP = mybir.AluOpType
AF = mybir.ActivationFunctionType

# i-blocks per batch: (start, len)
IBLOCKS = [(0, 128), (128, T - 128)]
DEBUG = False

# all small weights packed host-side into one [128, N] f32 tensor -> 1 DMA
WPACK_LAYOUT = [
    ("Wq", 128, 128), ("Wk", 128, 128), ("Wv", 128, 128), ("Wo", 128, 128),
    ("bq", 128, 1), ("bk", 128, 1),
    ("rkw0", 128, 32), ("rkw1", 128, 32),
    ("rkb0", 32, 1), ("rkb1", 32, 1), ("rbias0", 32, 1), ("rbias1", 32, 1),
    ("aqw0", 128, 32), ("aqw1", 128, 32), ("akw0", 128, 32), ("akw1", 128, 32),
    ("aqb0", 32, 1), ("aqb1", 32, 1), ("akb0", 32, 1), ("akb1", 32, 1),
    ("bvb", 128, 128), ("bob", 128, 128),
    ("rkw8_0", 128, 16), ("rkw8_1", 128, 16),
]
WPACK_OFF = {}
_o = 0
for _nm, _r, _c in WPACK_LAYOUT:
    WPACK_OFF[_nm] = _o
    _o += _c
WPACK_COLS = _o
WPACK_ROWS = (_o + 15) // 16 * 16
# one host-packed [IN_ROWS, 128] bf16 tensor: weights + q/k/v + absk + masks,
# loaded with a single xbar transpose-DMA
IN_OFF_W = 0
IN_OFF_Q = WPACK_ROWS
IN_OFF_K = IN_OFF_Q + 400
IN_OFF_V = IN_OFF_K + 400
IN_OFF_A0 = IN_OFF_V + 400
IN_OFF_A1 = IN_OFF_A0 + 400
IN_OFF_M = IN_OFF_A1 + 400
IN_ROWS = IN_OFF_M + 4 * 208


def chunks_for(il):
    """i-chunks (local_start, G) for the transpose-DMA stream."""
    out, i = [], 0
    while i < il:
        g = min(32, il - i)
        out.append((i, g))
        i += g
    return out


def build_kernel(ctx: ExitStack, tc: tile.TileContext, io: dict):
    nc = tc.nc

    relk = io["rel_kernel"]    # [2, 2, 200, 200, 128] bf16
    out = io["out"]            # [2, 200, 128]

    consts = ctx.enter_context(tc.tile_pool(name="consts", bufs=1))
    prep = ctx.enter_context(tc.tile_pool(name="prep", bufs=2))
    keep = ctx.enter_context(tc.tile_pool(name="keep", bufs=1))

    # Every input load is a transpose-DMA on the sync ring: Tile serializes
    # DMA-transposes against any concurrent plain DMA (xbar-deadlock guard),
    # so a single all-transpose FIFO stream is the only gap-free shape.

    ident = consts.tile([128, 128], F32, tag="ident")
    make_identity(nc, ident)

    inp = consts.tile([128, IN_ROWS], BF16, tag="inpack")
    nc.sync.dma_start_transpose(inp, io["inpack"])
    wtile = inp[:, :WPACK_ROWS]

    def wslice(nm, rows=128):
        o = WPACK_OFF[nm]
        c = dict((n, cc) for n, _r, cc in WPACK_LAYOUT)[nm]
        return wtile[:rows, o : o + c]

    rkt_pool = ctx.enter_context(tc.tile_pool(name="rkt", bufs=8))
    p4sb_pool = ctx.enter_context(tc.tile_pool(name="p4sb", bufs=66))
    sm = ctx.enter_context(tc.tile_pool(name="sm", bufs=2))
    ps_p4 = ctx.enter_context(tc.tile_pool(name="ps_p4", bufs=2, space="PSUM"))
    ps_s2 = ctx.enter_context(tc.tile_pool(name="ps_s2", bufs=1, space="PSUM"))
    ps_tp = ctx.enter_context(tc.tile_pool(name="ps_tp", bufs=1, space="PSUM"))
    ps_s1 = ctx.enter_context(tc.tile_pool(name="ps_s1", bufs=1, space="PSUM"))
    ps_x = ctx.enter_context(tc.tile_pool(name="ps_x", bufs=1, space="PSUM"))
    ps_y = ctx.enter_context(tc.tile_pool(name="ps_y", bufs=1, space="PSUM"))

    lhsT3 = {}
    c_sb = {}
    _cnt = [0]
    stream_state = {}
    if True:

        # ---- stream prerequisites first ----
        wq = wslice("Wq")
        bq_s = consts.tile([128, 1], F32, tag="bq_s")
        nc.scalar.activation(bq_s, wslice("bq"), AF.Copy, scale=SCALE)

        rkw16 = {}
        small_cols = {}
        for hr in range(H_REL):
            rkw16[hr] = wslice(f"rkw8_{hr}").bitcast(F8)
            t32 = consts.tile([DK, 1], F32, tag=f"rkb32_{hr}")
            nc.scalar.copy(t32, wslice(f"rkb{hr}", rows=DK))
            small_cols[("rkb", hr)] = t32
            ts_ = consts.tile([DK, 1], F32, tag=f"rbias_s{hr}")
            nc.scalar.activation(ts_, wslice(f"rbias{hr}", rows=DK), AF.Copy,
                                 scale=SCALE)
            small_cols[("rbias_s", hr)] = ts_

        # Pre-create lhsT3 tiles (filled later in prep); their C-matmuls
        # are emitted in the main loop, after the fill, so deps stay valid.
        for hr in range(H_REL):
            for b in range(BPC):
                for ib in range(2):
                    lhsT3[(hr, b, ib)] = keep.tile(
                        [128, 1024], BF16, tag=f"lt{hr}_{b}_{ib}",
                        name=f"lt{hr}_{b}_{ib}")

        def emit_stream(b, ib):
            """Emit one block's transpose-DMAs + stage-B matmuls + copies.
            Stage-C matmuls are deferred (pending_c) so the PE stream has no
            sem-wait bubbles; for block (0,0) the whole block defers (its
            lhsT3 is filled by prep, which runs concurrently), while later
            blocks flush one chunk behind to shorten the final tail."""
            i0, il = IBLOCKS[ib]
            st = {"s2ps": {}, "pending_c": []}
            defer_all = (b == 0 and ib == 0)
            for hr in range(H_REL):
                s2t = ps_s2.tile([128, T], F32, tag=f"s2h{hr}",
                                 name=f"s2h{hr}")
                st["s2ps"][hr] = s2t
                rkflat = relk[hr, b].flatten_outer_dims()  # [40000, 128]
                for ci, (ic0, G) in enumerate(chunks_for(il)):
                    rows = G * 100
                    rkt = rkt_pool.tile([128, 3200], BF16, tag="rkt",
                                        name="rkt")
                    # single HWDGE ring for transposes: alternating rings
                    # corrupts reads (cross-ring completion ordering)
                    nc.sync.dma_start_transpose(
                        rkt[:, :rows],
                        rkflat[(i0 + ic0) * 100 : (i0 + ic0 + G) * 100, :])
                    rkt8 = rkt.bitcast(F8)  # fp8 cols: 2m->j=m, 2m+1->j=100+m
                    if not defer_all:
                        # flush previous chunk's stage-C (copies long done)
                        for (o_, l_, r_, st_, sp_, tp_) in st["pending_c"]:
                            nc.tensor.matmul(o_, l_, r_, start=st_, stop=sp_,
                                             skip_group_check=True,
                                             tile_position=tp_)
                        st["pending_c"].clear()
                    for gl in range(G // 4):
                        gi = (ic0 + gl * 4) // 4   # group idx within block
                        p4 = ps_p4.tile([128, T], F32, tag="p4", name="p4")
                        for g in range(4):
                            nc.tensor.matmul(
                                p4[32 * g : 32 * (g + 1), :],
                                rkw16[hr],
                                rkt8[:, (gl * 4 + g) * T : (gl * 4 + g + 1) * T],
                                skip_group_check=True,
                                tile_position=(0, 96) if g == 3 else None)
                        p4c = p4sb_pool.tile([128, T], BF16, tag="p4sb",
                                             name="p4sb")
                        _cnt[0] += 1
                        if _cnt[0] % 2 == 0:
                            nc.vector.tensor_copy(p4c, p4)
                        else:
                            nc.scalar.copy(p4c, p4)
                        if DEBUG and hr == 0 and b == 0 and ib == 0:
                            nc.gpsimd.dma_start(io["dbg_p4"][gi], p4c)
                        bnd, k = gi // 8, gi % 8
                        Mb = min(32, il - 32 * bnd)
                        klast = (Mb + 3) // 4 - 1
                        S = bnd * 256 + k * 32
                        st["pending_c"].append((
                            s2t[32 * bnd : 32 * bnd + Mb, :],
                            lhsT3[(hr, b, ib)][:, S : S + Mb],
                            p4c, k == 0, k == klast,
                            (0, 96) if bnd == 3 else None))
            stream_state[(b, ib)] = st

        # hoist block (0,0)'s whole B phase ahead of the rest of prep: its
        # DMAs + stage-B matmuls only need rkw16, so the ring and PE start
        # immediately instead of stalling behind the prep dependency chain
        # everything arrives in the single inpack transpose; just slice
        xqT = inp[:, IN_OFF_Q : IN_OFF_Q + 400]
        xkT_pre = inp[:, IN_OFF_K : IN_OFF_K + 400]
        xvT_pre = inp[:, IN_OFF_V : IN_OFF_V + 400]
        akT_pre = {0: inp[:, IN_OFF_A0 : IN_OFF_A0 + 400],
                   1: inp[:, IN_OFF_A1 : IN_OFF_A1 + 400]}
        mtile_pre = {}
        for b in range(BPC):
            for ib in range(2):
                o = IN_OFF_M + (b * 2 + ib) * 208
                mtile_pre[(b, ib)] = inp[:, o : o + 208]

        emit_stream(0, 0)

        qT = {}
        for h in (H_ABS, H_ABS + 1, 0, 1):  # rel heads first
            qp = ps_p4.tile([DK, TT], F32, tag="p4", name="p4")
            nc.tensor.matmul(qp, wq[:, DK * h : DK * (h + 1)], xqT,
                             skip_group_check=True)
            t = keep.tile([DK, TT], F32, tag=f"qT{h}", name=f"qT{h}")
            nc.scalar.activation(t, qp, AF.Identity,
                                 bias=bq_s[DK * h : DK * (h + 1)], scale=SCALE)
            qT[h] = t

        qrbT = {}
        for hr in range(H_REL):
            t = keep.tile([DK, TT], F32, tag=f"qrbT{hr}", name=f"qrbT{hr}")
            nc.vector.tensor_scalar(t, qT[H_ABS + hr],
                                    small_cols[("rbias_s", hr)], None, OP.add)
            qrbT[hr] = t

        # blockmask[p, g] = 1 if p//32 == g
        bmask = consts.tile([128, 4], F32, tag="bmask")
        nc.vector.memset(bmask, 0.0)
        for g in range(4):
            nc.vector.memset(bmask[32 * g : 32 * (g + 1), g : g + 1], 1.0)

        # per-(hr, b, ib): c_sb (row constant) and the sparse stage-C lhsT
        for hr in range(H_REL):
            for b in range(BPC):
                for ib, (i0, il) in enumerate(IBLOCKS):
                    tsl = slice(b * T + i0, b * T + i0 + il)
                    cp = ps_s1.tile([128, 1], F32, tag="s1", name="s1")
                    nc.tensor.matmul(cp[:il, :], qrbT[hr][:, tsl],
                                     small_cols[("rkb", hr)],
                                     skip_group_check=True)
                    t = keep.tile([128, 1], F32, tag=f"c{hr}_{b}_{ib}",
                                  name=f"c{hr}_{b}_{ib}")
                    nc.scalar.copy(t[:il, :], cp[:il, :])
                    c_sb[(hr, b, ib)] = t

                    n_gi = il // 4
                    # q'pack[32g+o, gi] = qrb[o, t(gi*4+g)] via 4 PE matmuls
                    qp_ps = ps_s1.tile([128, 32], F32, tag="s1",
                                       name="s1")
                    for g in range(4):
                        s0 = b * T + i0 + g
                        src = qrbT[hr][:, s0 : s0 + 4 * (n_gi - 1) + 1 : 4]
                        nc.tensor.matmul(
                            qp_ps[32 * g : 32 * (g + 1), :n_gi],
                            ident[:DK, :DK], src, skip_group_check=True,
                            tile_position=(0, 96) if g == 3 else None)
                    qpk = prep.tile([128, 32], F32, tag="qpk", name="qpk")
                    nc.vector.tensor_copy(qpk[:, :n_gi], qp_ps[:, :n_gi])

                    # zero-interleaved block-diag lhsT: group gi=(bnd,k)'s
                    # 4 diag columns live at col bnd*256 + k*36 (+g); the
                    # matmul window for (bnd,k) is [bnd*256+k*32, +Mb)
                    lt = lhsT3[(hr, b, ib)]
                    nc.vector.memset(lt, 0.0)
                    nb = il // 32          # full 32-i bands
                    rem = (il - 32 * nb) // 4   # leftover 4-i groups
                    rs_l = lt.ap[0][0]
                    rs_q = qpk.ap[0][0]
                    rs_m = bmask.ap[0][0]
                    if nb:
                        o4 = bass.AP(tensor=lt.tensor, offset=lt.offset,
                                     ap=[[rs_l, 128], [256, nb], [36, 8], [1, 4]])
                        i4 = bass.AP(tensor=qpk.tensor, offset=qpk.offset,
                                     ap=[[rs_q, 128], [8, nb], [1, 8], [0, 4]])
                        m4 = bass.AP(tensor=bmask.tensor, offset=bmask.offset,
                                     ap=[[rs_m, 128], [0, nb], [0, 8], [1, 4]])
                        nc.vector.tensor_tensor(o4, i4, m4, op=OP.mult)
                    if rem:
                        o4 = bass.AP(tensor=lt.tensor,
                                     offset=lt.offset + nb * 256,
                                     ap=[[rs_l, 128], [36, rem], [1, 4]])
                        i4 = bass.AP(tensor=qpk.tensor,
                                     offset=qpk.offset + nb * 8,
                                     ap=[[rs_q, 128], [1, rem], [0, 4]])
                        m4 = bass.AP(tensor=bmask.tensor, offset=bmask.offset,
                                     ap=[[rs_m, 128], [0, rem], [1, 4]])
                        nc.vector.tensor_tensor(o4, i4, m4, op=OP.mult)

        # ---- rest of prep: deferred until after the first block's stream
        # emission so ~35 slow SWDGE small-DMAs and their dependent PE ops
        # don't sit ahead of the stream matmuls in the PE's in-order queue
        # (head-of-line blocking that stalled the DMA ring ~11us/chunk) ----
        kT = {}
        vb = {}
        qaT = {}
        kaT = {}
        mb = {}
        reph = {}

        def emit_rest_prep():
            xkT = xkT_pre
            xvT = xvT_pre

            wk = wslice("Wk")
            wv = wslice("Wv")
            wo32 = consts.tile([128, 128], F32, tag="wo32")
            nc.scalar.copy(wo32, wslice("Wo"))
            wo = wo32
            bk_c = consts.tile([128, 1], F32, tag="bk32")
            nc.scalar.copy(bk_c, wslice("bk"))
            bv_b = consts.tile([128, 128], F32, tag="bvb32")
            nc.scalar.copy(bv_b, wslice("bvb"))
            bo_b = consts.tile([128, 128], F32, tag="bob32")
            nc.scalar.copy(bo_b, wslice("bob"))

            abs_w = {}
            for hh in range(H_ABS):
                abs_w[("aqw", hh)] = wslice(f"aqw{hh}")
                abs_w[("akw", hh)] = wslice(f"akw{hh}")
                akb32 = consts.tile([DK, 1], F32, tag=f"akb32_{hh}")
                nc.scalar.copy(akb32, wslice(f"akb{hh}", rows=DK))
                small_cols[("akb", hh)] = akb32
                ts_ = consts.tile([DK, 1], F32, tag=f"aqb_s{hh}")
                nc.scalar.activation(ts_, wslice(f"aqb{hh}", rows=DK), AF.Copy,
                                     scale=SCALE)
                small_cols[("aqb_s", hh)] = ts_

            for h in range(H):
                kp = ps_p4.tile([DK, TT], F32, tag="p4", name="p4")
                nc.tensor.matmul(kp, wk[:, DK * h : DK * (h + 1)], xkT,
                                 skip_group_check=True)
                t = keep.tile([DK, TT], F32, tag=f"kT{h}", name=f"kT{h}")
                nc.scalar.activation(t, kp, AF.Identity,
                                     bias=bk_c[DK * h : DK * (h + 1)])
                kT[h] = t

            for b in range(BPC):
                for jb, (j0, jl) in enumerate(IBLOCKS):
                    # value tokens are host-permuted into fp8-perm j order,
                    # so plain contiguous slices line up with pT rows
                    vp = ps_s1.tile([128, 128], F32, tag="s1", name="s1")
                    nc.tensor.matmul(vp[:jl, :],
                                     xvT[:, b * T + j0 : b * T + j0 + jl], wv,
                                     skip_group_check=True)
                    t = keep.tile([128, 128], F32, tag=f"v{b}_{jb}", name=f"v{b}_{jb}")
                    nc.vector.tensor_add(t[:jl, :], vp[:jl, :], bv_b[:jl, :])
                    vb[(b, jb)] = t

            for hh in range(H_ABS):
                akT = akT_pre[hh]
                pp = ps_p4.tile([DK, TT], F32, tag="p4", name="p4")
                nc.tensor.matmul(pp, abs_w[("aqw", hh)], akT, skip_group_check=True)
                pqT = prep.tile([DK, TT], F32, tag="pqT", name="pqT")
                nc.scalar.activation(pqT, pp, AF.Identity,
                                     bias=small_cols[("aqb_s", hh)], scale=SCALE)
                t = keep.tile([DK, TT], F32, tag=f"qaT{hh}", name=f"qaT{hh}")
                nc.vector.tensor_add(t, qT[hh], pqT)
                qaT[hh] = t

                pp2 = ps_p4.tile([DK, TT], F32, tag="p4", name="p4")
                nc.tensor.matmul(pp2, abs_w[("akw", hh)], akT, skip_group_check=True)
                pkT = prep.tile([DK, TT], F32, tag="pqT", name="pqT")
                nc.scalar.activation(pkT, pp2, AF.Identity,
                                     bias=small_cols[("akb", hh)])
                t = keep.tile([DK, TT], F32, tag=f"kaT{hh}", name=f"kaT{hh}")
                nc.vector.tensor_add(t, kT[hh], pkT)
                kaT[hh] = t

            for b in range(BPC):
                for ib, (i0, il) in enumerate(IBLOCKS):
                    mi = mtile_pre[(b, ib)]
                    t = keep.tile([128, T], F32, tag=f"mb{b}_{ib}", name=f"mb{b}_{ib}")
                    nc.vector.tensor_scalar(t[:il, :], mi[:il, :T], 1e9, -1e9,
                                            OP.mult, OP.add)
                    mb[(b, ib)] = t

            reph["wo"] = wo
            reph["bo_b"] = bo_b

        emit_rest_prep()

    # ---------------- main phase ----------------

    out_stores = []
    for b in range(BPC):
        for ib, (i0, il) in enumerate(IBLOCKS):
            if (b, ib) not in stream_state:
                emit_stream(b, ib)
            sstate = stream_state[(b, ib)]
            s2ps = sstate["s2ps"]
            for (ap_out, ap_l, ap_r, st_, sp_, tp_) in sstate["pending_c"]:
                nc.tensor.matmul(ap_out, ap_l, ap_r, start=st_, stop=sp_,
                                 skip_group_check=True, tile_position=tp_)
            sstate["pending_c"].clear()

            if DEBUG and b == 0 and ib == 0:
                for hr_ in range(H_REL):
                    s2dump = sm.tile([128, T], F32, tag="s2dump",
                                     name="s2dump")
                    nc.vector.tensor_copy(s2dump[:il, :], s2ps[hr_][:il, :])
                    nc.gpsimd.dma_start(io["dbg_s2"][hr_, :il, :],
                                        s2dump[:il, :])

            # ---- epilogue: scores -> softmax -> p@v -> out linear ----
            tsl = slice(b * T + i0, b * T + i0 + il)
            bsl = slice(b * T, (b + 1) * T)
            xT_ps = ps_x.tile([128, 128], F32, tag="xT", name="xT")
            last_blk = (b == BPC - 1 and ib == 1)
            horder = (0, 1, 2, 3) if last_blk else (2, 3, 0, 1)
            for h in horder:  # rel first frees s2 early; last block abs
                              # first overlaps them with its rel stream
                is_rel = h >= H_ABS
                st = sm.tile([128, T], F32, tag="st", name="st")
                if is_rel:
                    hr = h - H_ABS
                    kTp = bass.AP(tensor=kT[h].tensor,
                                  offset=kT[h].offset + b * T,
                                  ap=[[kT[h].ap[0][0], DK], [1, 100], [100, 2]])
                    nc.tensor.matmul(s2ps[hr][:il, :], qT[h][:, tsl],
                                     kTp, start=False, stop=True,
                                     skip_group_check=True)
                    nc.vector.tensor_scalar(st[:il, :], s2ps[hr][:il, :],
                                            c_sb[(hr, b, ib)][:il], None,
                                            OP.add)
                    nc.vector.tensor_add(st[:il, :], st[:il, :],
                                         mb[(b, ib)][:il, :])
                else:
                    s1 = ps_s1.tile([128, T], F32, tag="s1", name="s1")
                    kaTp = bass.AP(tensor=kaT[h].tensor,
                                   offset=kaT[h].offset + b * T,
                                   ap=[[kaT[h].ap[0][0], DK], [1, 100],
                                       [100, 2]])
                    nc.tensor.matmul(s1[:il, :], qaT[h][:, tsl],
                                     kaTp, skip_group_check=True)
                    nc.vector.tensor_add(st[:il, :], s1[:il, :],
                                         mb[(b, ib)][:il, :])

                nmax = sm.tile([128, 1], F32, tag="nmax", name="nmax")
                nc.vector.tensor_reduce(nmax[:il], st[:il, :], AX.X, OP.max,
                                        negate=True)
                p = sm.tile([128, T], F32, tag="p", name="p")
                rsum = sm.tile([128, 1], F32, tag="rsum", name="rsum")
                nc.scalar.activation(p[:il, :], st[:il, :], AF.Exp,
                                     bias=nmax[:il], accum_out=rsum[:il])
                rcp = sm.tile([128, 1], F32, tag="rcp", name="rcp")
                nc.vector.reciprocal(rcp[:il], rsum[:il])
                nc.vector.tensor_scalar(p[:il, :], p[:il, :], rcp[:il], None,
                                        OP.mult)
                if DEBUG:
                    nc.gpsimd.dma_start(
                        io["dbg_st"][b, ib, h, :il, :], st[:il, :])

                hsl = slice(DK * h, DK * (h + 1))
                for jb, (j0, jl) in enumerate(IBLOCKS):
                    tp = ps_tp.tile([128, 128], F32, tag="tp", name="tp")
                    nc.tensor.matmul(tp[:jl, :il], p[:il, j0 : j0 + jl],
                                     ident[:il, :il], is_transpose=True,
                                     skip_group_check=True)
                    pT = sm.tile([128, 128], F32, tag="pT", name="pT")
                    nc.scalar.copy(pT[:jl, :il], tp[:jl, :il])
                    nc.tensor.matmul(xT_ps[hsl, :il], vb[(b, jb)][:jl, hsl],
                                     pT[:jl, :il],
                                     start=(jb == 0), stop=(jb == 1),
                                     skip_group_check=True,
                                     tile_position=(0, 96) if h == 3 else None)

            xT_sb = sm.tile([128, 128], F32, tag="xT_sb", name="xT_sb")
            nc.scalar.copy(xT_sb[:, :il], xT_ps[:, :il])
            y_ps = ps_y.tile([128, 128], F32, tag="y", name="y")
            nc.tensor.matmul(y_ps[:il, :], xT_sb[:, :il], reph["wo"],
                             skip_group_check=True)
            y_sb = keep.tile([128, 128], F32, tag=f"y_out{b}_{ib}",
                             name=f"y_out{b}_{ib}")
            nc.vector.tensor_add(y_sb[:il, :], y_ps[:il, :],
                                 reph["bo_b"][:il, :])
            out_stores.append((out[b, i0 : i0 + il, :], y_sb[:il, :]))

    # output stores after the whole stream: any plain DMA issued mid-stream
    # serializes against the transpose DMAs (Tile xbar guard)
    for dst, srct in out_stores:
        nc.scalar.dma_start(dst, srct)


def build_nc():
    nc = bacc.Bacc(trn_type="TRN2")
    io = {}
    io["inpack"] = nc.dram_tensor(
        "inpack", [IN_ROWS, 128], BF16, kind="ExternalInput").ap()
    io["rel_kernel"] = nc.dram_tensor(
        "rel_kernel", [H_REL, BPC, T, 100, D], BF16, kind="ExternalInput"
    ).ap()
    io["out"] = nc.dram_tensor("out", [BPC, T, D], F32, kind="ExternalOutput").ap()
    if DEBUG:
        io["dbg_st"] = nc.dram_tensor(
            "dbg_st", [BPC, 2, H, 128, T], F32, kind="ExternalOutput").ap()
        io["dbg_p4"] = nc.dram_tensor(
            "dbg_p4", [32, 128, T], BF16, kind="ExternalOutput").ap()
        io["dbg_s2"] = nc.dram_tensor(
            "dbg_s2", [H_REL, 128, T], F32, kind="ExternalOutput").ap()

    with tile.TileContext(nc) as tc:
        with ExitStack() as ctx:
            build_kernel(ctx, tc, io)
    nc.compile()
    return nc


_NC_CACHE = None


def _get_nc():
    global _NC_CACHE
    if _NC_CACHE is None:
        _NC_CACHE = build_nc()
    return _NC_CACHE


def make_in_maps(inputs):
    """Shard full inputs into per-core input maps."""
    f32 = np.float32
    g = {k: np.asarray(inputs[k], dtype=f32) for k in
         ["Wq", "bq", "Wk", "bk", "Wv", "bv", "abs_q_w", "abs_q_b",
          "abs_k_w", "abs_k_b", "rel_k_w", "rel_k_b", "rel_bias",
          "Wo", "bo"]}
    wp = np.zeros((128, WPACK_COLS), f32)

    def put(nm, arr):
        o = WPACK_OFF[nm]
        arr = np.asarray(arr, f32)
        if arr.ndim == 1:
            arr = arr[:, None]
        wp[: arr.shape[0], o : o + arr.shape[1]] = arr

    put("Wq", g["Wq"]); put("Wk", g["Wk"]); put("Wv", g["Wv"])
    put("Wo", g["Wo"]); put("bq", g["bq"]); put("bk", g["bk"])
    for hr in range(H_REL):
        put(f"rkw{hr}", g["rel_k_w"][hr])
        put(f"rkb{hr}", g["rel_k_b"][hr])
        put(f"rbias{hr}", g["rel_bias"][0, hr, 0, :])
    for hh in range(H_ABS):
        put(f"aqw{hh}", g["abs_q_w"][hh])
        put(f"akw{hh}", g["abs_k_w"][hh])
        put(f"aqb{hh}", g["abs_q_b"][hh])
        put(f"akb{hh}", g["abs_k_b"][hh])
    put("bvb", np.tile(g["bv"][None, :], (128, 1)))
    put("bob", np.tile(g["bo"][None, :], (128, 1)))
    import ml_dtypes
    bf = ml_dtypes.bfloat16
    # wpack stored transposed (host) so an xbar transpose-DMA yields [128, N]
    wpT = np.zeros((WPACK_ROWS, 128), np.float32)
    wpT[:WPACK_COLS, :] = wp.T
    weights = {}
    query = np.asarray(inputs["query"], dtype=f32).astype(bf)
    key = np.asarray(inputs["key"], dtype=f32).astype(bf)
    value = np.asarray(inputs["value"], dtype=f32).astype(bf)
    # mask pre-transposed+padded on host, j in fp8-perm order
    # perm(2m)=m, perm(2m+1)=100+m
    perm = np.empty(T, np.int64)
    perm[0::2] = np.arange(100)
    perm[1::2] = np.arange(100) + 100
    mask_i = np.asarray(inputs["mask"], dtype=np.int32)[:, 0]  # [B, T, T]
    maskT = np.zeros((B, 2, 208, 128), f32)
    for ib, (i0, il) in enumerate([(0, 128), (128, 72)]):
        maskT[:, ib, :T, :il] = mask_i[:, i0:i0+il, :][:, :, perm].transpose(0, 2, 1)
    maskT = maskT.astype(bf)
    # rel_kernel: fp8 e4m3, byte-pair packed (RK[i,j,d], RK[i,j+100,d]) per
    # 16-bit unit -> after xbar transpose partition=d, fp8 col order = perm
    rk8 = np.asarray(inputs["rel_kernel"], dtype=f32).astype(
        ml_dtypes.float8_e4m3fn)
    u8 = np.empty(rk8.shape[:3] + (100, 256), np.uint8)
    u8[..., 0::2] = rk8[:, :, :, :100, :].view(np.uint8)
    u8[..., 1::2] = rk8[:, :, :, 100:, :].view(np.uint8)
    relk = u8.view(np.uint16).view(bf)  # [H, B, 200, 100, 128] bf16-bits
    absk = np.asarray(inputs["abs_kernel"], dtype=f32).astype(bf)

    in_maps = []
    for c in range(N_CORES):
        bs = slice(c * BPC, (c + 1) * BPC)
        m = dict(weights)
        ip = np.zeros((IN_ROWS, 128), np.float32)
        ip[:WPACK_ROWS] = wpT.astype(np.float32)
        ip[IN_OFF_Q : IN_OFF_Q + 400] = query[bs].reshape(400, 128)
        ip[IN_OFF_K : IN_OFF_K + 400] = key[bs].reshape(400, 128)
        ip[IN_OFF_V : IN_OFF_V + 400] = value[bs][:, perm, :].reshape(400, 128)
        ip[IN_OFF_A0 : IN_OFF_A0 + 400] = absk[0, bs].reshape(400, 128)
        ip[IN_OFF_A1 : IN_OFF_A1 + 400] = absk[1, bs].reshape(400, 128)
        ip[IN_OFF_M : IN_OFF_M + 832] = maskT[bs].reshape(832, 128)
        ip_bf = ip.astype(bf)
        for hr in range(H_REL):
            rw8 = g["rel_k_w"][hr].astype(ml_dtypes.float8_e4m3fn)  # [128,32]
            units = rw8.view(np.uint8).reshape(128, 16, 2)
            u16 = (units[:, :, 0].astype(np.uint16)
                   | (units[:, :, 1].astype(np.uint16) << 8))  # [128,16]
            o = WPACK_OFF[f"rkw8_{hr}"]
            # inpack row (IN_OFF_W + o + m), col d  <- unit[d, m]
            ip_bf[o : o + 16, :] = u16.T.view(bf).reshape(16, 128)
        m["inpack"] = np.ascontiguousarray(ip_bf)
        m["rel_kernel"] = np.ascontiguousarray(relk[:, bs])
        in_maps.append(m)
    return in_maps


def kernel(**inputs) -> np.ndarray:
    nc = _get_nc()
    in_maps = make_in_maps(inputs)
    res = run_bass_kernel_spmd(nc, in_maps, core_ids=list(range(N_CORES)))
    return np.concatenate([r["out"] for r in res.results], axis=0)


if __name__ == "__main__":
    nc = build_nc()
    print("built ok")



# revision 7
# speedup vs baseline: 1.7257x; 1.7257x over previous
"""Trainium2 Bass kernel for nn_Attention_12463995093474 (sparse_attention).

Math (reference):
  q/k/v = content linears; 2 absolute heads, 2 relative heads (DK=32).
  abs:  scores = (Xq_a + abs_kernel@abs_q_w) @ (Xk_a + abs_kernel@abs_k_w)^T
  rel:  scores = Xq_r @ Xk_r^T + (Xq_r + rel_bias) . (rel_kernel@rel_k_w + rel_k_b)
  softmax(mask) @ v -> out linear.

Key algebraic collapse: the dominant rel term
    s2[i,j] = sum_o q''[i,o] * (sum_d RK[i,j,d] W[d,o])  (+ c[i])
            = sum_d qW[i,d] * RK[i,j,d],   qW = q'' @ W^T
so RK contracts DIRECTLY against a per-row vector (32x fewer MACs than
materializing R).  Implementation:
  - rel_kernel is pre-transposed on host to [h, b, d, i, j] fp8(e4m3)
    so it streams with PLAIN DMAs (no xbar transpose) at full HBM bw.
  - per (head h, row i) one PE matmul: lhsT = qW8 fp8 columns, rhs =
    RK[i]^T fp8 [128 x 200] -> 200 cols @ 1 cycle/row.  PSUM rows must
    land 32-aligned, so rows are processed DESCENDING within each
    32-row band: matmul for row r writes rows [band..band+r]
    (start=True); garbage in rows < r is overwritten by later matmuls,
    leaving every row's true matvec in place.
  - content scores accumulate into the same PSUM tile (start=False),
    then softmax -> p@v -> output linear, all matmuls in bf16.
  - qW is scaled by 64 on-device before the fp8 cast (values sigma
    ~0.005 would be subnormal in e4m3); the 2^-6 is folded into the
    epilogue's existing (s2 + c) * scale DVE op.

Sharding: data-parallel over batch, B=16 -> 2 batches per core on 8 cores.
"""

import numpy as np
from contextlib import ExitStack

import concourse.bass as bass
import concourse.bacc as bacc
import concourse.tile as tile
from concourse import mybir
from concourse.masks import make_identity
from concourse.bass_utils import run_bass_kernel_spmd

B, T, D = 16, 200, 128
H_ABS, H_REL, H, DK = 2, 2, 4, 32
N_CORES = 8
BPC = B // N_CORES  # batches per core
SCALE = 1.0 / float(DK) ** 0.5
TT = BPC * T  # tokens per core (400)
UP = 64.0     # qW fp8 upscale (power of 2)

F32 = mybir.dt.float32
BF16 = mybir.dt.bfloat16
F8 = mybir.dt.float8e4
AX = mybir.AxisListType
OP = mybir.AluOpType
AF = mybir.ActivationFunctionType

# i-blocks per batch: (start, len)
IBLOCKS = [(0, 128), (128, T - 128)]

# packed weights, stored [128, col] on host; loaded with one plain DMA
WPACK_LAYOUT = [
    ("Wq", 128, 128), ("Wk", 128, 128), ("Wv", 128, 128), ("Wo", 128, 128),
    ("bq", 128, 1), ("bk", 128, 1),
    ("rkwT0", 32, 128), ("rkwT1", 32, 128),
    ("rkb0", 32, 1), ("rkb1", 32, 1), ("rbias0", 32, 1), ("rbias1", 32, 1),
    ("aqw0", 128, 32), ("aqw1", 128, 32), ("akw0", 128, 32), ("akw1", 128, 32),
    ("aqb0", 32, 1), ("aqb1", 32, 1), ("akb0", 32, 1), ("akb1", 32, 1),
    ("bvb", 128, 128), ("bob", 128, 128),
]
WPACK_OFF = {}
_o = 0
for _nm, _r, _c in WPACK_LAYOUT:
    WPACK_OFF[_nm] = _o
    _o += _c
WPACK_COLS = _o
# one host-packed [128, IN_COLS] bf16 tensor: weights + q/k/v + absk + masks
IN_OFF_W = 0
IN_OFF_Q = WPACK_COLS
IN_OFF_K = IN_OFF_Q + TT
IN_OFF_V = IN_OFF_K + TT
IN_OFF_A0 = IN_OFF_V + TT
IN_OFF_A1 = IN_OFF_A0 + TT
IN_OFF_M = IN_OFF_A1 + TT
IN_COLS = IN_OFF_M + 4 * T

DEBUG = False


def chunks_for(il):
    """i-chunks (local_start, G) for the RK stream; 32-aligned bands."""
    out, i = [], 0
    while i < il:
        g = min(32, il - i)
        out.append((i, g))
        i += g
    return out


def build_kernel(ctx: ExitStack, tc: tile.TileContext, io: dict):
    nc = tc.nc

    relk = io["rel_kernel"]    # [2, 2, 128, 200, 100] bf16 carrier of fp8
    out = io["out"]            # [2, 200, 128]

    consts = ctx.enter_context(tc.tile_pool(name="consts", bufs=1))
    keep = ctx.enter_context(tc.tile_pool(name="keep", bufs=1))
    prep = ctx.enter_context(tc.tile_pool(name="prep", bufs=2))
    rkt_pool = ctx.enter_context(tc.tile_pool(name="rkt", bufs=10))
    sm = ctx.enter_context(tc.tile_pool(name="sm", bufs=2))
    ps_s2 = ctx.enter_context(tc.tile_pool(name="ps_s2", bufs=2, space="PSUM"))
    ps_pp = ctx.enter_context(tc.tile_pool(name="ps_pp", bufs=1, space="PSUM"))
    ps_x = ctx.enter_context(tc.tile_pool(name="ps_x", bufs=1, space="PSUM"))
    ps_ty = ctx.enter_context(tc.tile_pool(name="ps_ty", bufs=2, space="PSUM"))

    # ---------------- input load (plain DMAs only) ----------------
    inp = consts.tile([128, IN_COLS], BF16, tag="inpack")
    nc.gpsimd.dma_start(inp, io["inpack"])

    def wslice(nm, rows=128):
        o = WPACK_OFF[nm]
        c = dict((n, cc) for n, _r, cc in WPACK_LAYOUT)[nm]
        return inp[:rows, o : o + c]

    xqT = inp[:, IN_OFF_Q : IN_OFF_Q + TT]
    xkT = inp[:, IN_OFF_K : IN_OFF_K + TT]
    xvT = inp[:, IN_OFF_V : IN_OFF_V + TT]
    akT_pre = {0: inp[:, IN_OFF_A0 : IN_OFF_A0 + TT],
               1: inp[:, IN_OFF_A1 : IN_OFF_A1 + TT]}

    ident = consts.tile([128, 128], BF16, tag="ident")
    make_identity(nc, ident)

    # ---------------- prep ----------------
    wq = wslice("Wq")
    wk = wslice("Wk")
    wv = wslice("Wv")
    wo = wslice("Wo")

    bq_s = consts.tile([128, 1], F32, tag="bq_s")
    nc.scalar.activation(bq_s, wslice("bq"), AF.Copy, scale=SCALE)
    bq_s64 = consts.tile([128, 1], F32, tag="bq_s64")
    nc.scalar.activation(bq_s64, wslice("bq"), AF.Copy, scale=SCALE * UP)
    bk_c = consts.tile([128, 1], F32, tag="bk_c")
    nc.scalar.copy(bk_c, wslice("bk"))
    bv_b = consts.tile([128, 128], F32, tag="bv_b")
    nc.scalar.copy(bv_b, wslice("bvb"))
    bo_b = consts.tile([128, 128], F32, tag="bo_b")
    nc.scalar.copy(bo_b, wslice("bob"))

    small_cols = {}
    for hr in range(H_REL):
        ts_ = consts.tile([DK, 1], F32, tag=f"rbias_s64_{hr}")
        nc.scalar.activation(ts_, wslice(f"rbias{hr}", rows=DK), AF.Copy,
                             scale=SCALE * UP)
        small_cols[("rbias_s64", hr)] = ts_
    for hh in range(H_ABS):
        ts_ = consts.tile([DK, 1], F32, tag=f"aqb_s{hh}")
        nc.scalar.activation(ts_, wslice(f"aqb{hh}", rows=DK), AF.Copy,
                             scale=SCALE)
        small_cols[("aqb_s", hh)] = ts_
        akb32 = consts.tile([DK, 1], F32, tag=f"akb32_{hh}")
        nc.scalar.copy(akb32, wslice(f"akb{hh}", rows=DK))
        small_cols[("akb", hh)] = akb32

    # content q projections: abs heads scaled by SCALE, rel heads by SCALE*64
    qT = {}      # abs heads, bf16 [32, TT]
    qT64 = {}    # rel heads (x64), bf16 [32, TT]
    for h in range(H):
        qp = ps_pp.tile([DK, TT], F32, tag="pp", name="pp")
        nc.tensor.matmul(qp, wq[:, DK * h : DK * (h + 1)], xqT,
                         skip_group_check=True)
        t = keep.tile([DK, TT], BF16, tag=f"qT{h}", name=f"qT{h}")
        if h < H_ABS:
            nc.scalar.activation(t, qp, AF.Identity,
                                 bias=bq_s[DK * h : DK * (h + 1)], scale=SCALE)
            qT[h] = t
        else:
            nc.scalar.activation(t, qp, AF.Identity,
                                 bias=bq_s64[DK * h : DK * (h + 1)],
                                 scale=SCALE * UP)
            qT64[h - H_ABS] = t

    # qW8[hr]: fp8 [128 d, TT] = ((q*SCALE + rb*SCALE) @ W^T) * 64
    qW8 = {}
    qrb64 = {}
    for hr in range(H_REL):
        t = keep.tile([DK, TT], BF16, tag=f"qrb64_{hr}", name=f"qrb64_{hr}")
        nc.vector.tensor_scalar(t, qT64[hr], small_cols[("rbias_s64", hr)],
                                None, OP.add)
        qrb64[hr] = t
        qwp = ps_pp.tile([128, TT], F32, tag="pp", name="pp")
        nc.tensor.matmul(qwp, wslice(f"rkwT{hr}", rows=DK), t,
                         skip_group_check=True)
        q8 = keep.tile([128, TT], F8, tag=f"qW8_{hr}", name=f"qW8_{hr}")
        nc.vector.tensor_copy(q8, qwp)
        qW8[hr] = q8

    # content k projections (raw) bf16 [32, TT] per head
    kT = {}
    for h in range(H):
        kp = ps_pp.tile([DK, TT], F32, tag="pp", name="pp")
        nc.tensor.matmul(kp, wk[:, DK * h : DK * (h + 1)], xkT,
                         skip_group_check=True)
        t = keep.tile([DK, TT], BF16, tag=f"kT{h}", name=f"kT{h}")
        nc.scalar.activation(t, kp, AF.Identity,
                             bias=bk_c[DK * h : DK * (h + 1)])
        kT[h] = t

    # values, bf16 [jl, 128] per (b, jblock)
    vb = {}
    for b in range(BPC):
        for jb, (j0, jl) in enumerate(IBLOCKS):
            vp = ps_ty.tile([128, 128], F32, tag="ty", name="ty")
            nc.tensor.matmul(vp[:jl, :],
                             xvT[:, b * T + j0 : b * T + j0 + jl], wv,
                             skip_group_check=True)
            t = keep.tile([128, 128], BF16, tag=f"v{b}_{jb}",
                          name=f"v{b}_{jb}")
            nc.vector.tensor_add(t[:jl, :], vp[:jl, :], bv_b[:jl, :])
            vb[(b, jb)] = t

    # absolute-position heads: qaT = qT + Pq, kaT = kT + Pk (bf16 [32, TT])
    qaT = {}
    kaT = {}
    for hh in range(H_ABS):
        akT = akT_pre[hh]
        pp = ps_pp.tile([DK, TT], F32, tag="pp", name="pp")
        nc.tensor.matmul(pp, wslice(f"aqw{hh}"), akT, skip_group_check=True)
        pqT = prep.tile([DK, TT], BF16, tag="pqT", name="pqT")
        nc.scalar.activation(pqT, pp, AF.Identity,
                             bias=small_cols[("aqb_s", hh)], scale=SCALE)
        t = keep.tile([DK, TT], BF16, tag=f"qaT{hh}", name=f"qaT{hh}")
        nc.vector.tensor_add(t, qT[hh], pqT)
        qaT[hh] = t

        pp2 = ps_pp.tile([DK, TT], F32, tag="pp", name="pp")
        nc.tensor.matmul(pp2, wslice(f"akw{hh}"), akT, skip_group_check=True)
        pkT = prep.tile([DK, TT], BF16, tag="pqT", name="pqT")
        nc.scalar.activation(pkT, pp2, AF.Identity,
                             bias=small_cols[("akb", hh)])
        t = keep.tile([DK, TT], BF16, tag=f"kaT{hh}", name=f"kaT{hh}")
        nc.vector.tensor_add(t, kT[hh], pkT)
        kaT[hh] = t

    # masks: mb = mask*1e9 - 1e9, f32 [il, T]
    mb = {}
    for b in range(BPC):
        for ib, (i0, il) in enumerate(IBLOCKS):
            o = IN_OFF_M + (b * 2 + ib) * T
            t = keep.tile([128, T], F32, tag=f"mb{b}_{ib}", name=f"mb{b}_{ib}")
            nc.vector.tensor_scalar(t[:il, :], inp[:il, o : o + T], 1e9, -1e9,
                                    OP.mult, OP.add)
            mb[(b, ib)] = t

    # c64[i] = qrb64 . rkb  (carries the x64; f32 col per (hr, b, ib))
    c_sb = {}
    for hr in range(H_REL):
        for b in range(BPC):
            for ib, (i0, il) in enumerate(IBLOCKS):
                tsl = slice(b * T + i0, b * T + i0 + il)
                cp = ps_ty.tile([128, 1], F32, tag="ty", name="ty")
                nc.tensor.matmul(cp[:il, :], qrb64[hr][:, tsl],
                                 wslice(f"rkb{hr}", rows=DK),
                                 skip_group_check=True)
                t = keep.tile([128, 1], F32, tag=f"c{hr}_{b}_{ib}",
                              name=f"c{hr}_{b}_{ib}")
                nc.scalar.copy(t[:il, :], cp[:il, :])
                c_sb[(hr, b, ib)] = t

    # ---------------- main loop ----------------
    out_stores = []
    for b in range(BPC):
        for ib, (i0, il) in enumerate(IBLOCKS):
            bsl = slice(b * T, (b + 1) * T)
            tsl = slice(b * T + i0, b * T + i0 + il)
            s2ps = {}
            for hr in range(H_REL):
                # full-bank rows (512 f32 = 2048B) so per-row matmul writes
                # land bank-aligned; only [:, :T] is used
                s2t = ps_s2.tile([128, 512], F32, tag=f"s2h{hr}",
                                 name=f"s2h{hr}")
                s2ps[hr] = s2t
                # stream RK chunks + per-row matvec matmuls
                for (ic0, G) in chunks_for(il):
                    rkt = rkt_pool.tile([128, 3200], BF16, tag="rkt",
                                        name="rkt")
                    nc.gpsimd.dma_start(
                        rkt[:, : G * 100],
                        relk[hr, b][:, i0 + ic0 : i0 + ic0 + G, :])
                    rkt8 = rkt.bitcast(F8)
                    # 32-band at ic0 (chunks are band-aligned); descending
                    # rows: row r's matmul writes rows [0..r] of the band,
                    # start=True zeroes/overwrites garbage below it.
                    for r in range(G - 1, -1, -1):
                        gb = b * T + i0 + ic0
                        nc.tensor.matmul(
                            s2t[ic0 : ic0 + r + 1, :T],
                            qW8[hr][:, gb : gb + r + 1],
                            rkt8[:, r * T : (r + 1) * T],
                            start=True, stop=False,
                            skip_group_check=True,
                            tile_position=(0, ic0))
                # content scores accumulate on top
                nc.tensor.matmul(s2t[:il, :T], qT64[hr][:, tsl],
                                 kT[H_ABS + hr][:, bsl],
                                 start=False, stop=True,
                                 skip_group_check=True)

            # ---- epilogue: scores -> softmax -> p@v -> out linear ----
            xT_ps = ps_x.tile([128, 128], F32, tag="xT", name="xT")
            for h in range(H):
                is_rel = h >= H_ABS
                st = sm.tile([128, T], F32, tag="st", name="st")
                if is_rel:
                    hr = h - H_ABS
                    # st = (s2 + c64) * 2^-6 + mask
                    nc.vector.tensor_scalar(st[:il, :], s2ps[hr][:il, :T],
                                            c_sb[(hr, b, ib)][:il],
                                            1.0 / UP, OP.add, OP.mult)
                    nc.vector.tensor_add(st[:il, :], st[:il, :],
                                         mb[(b, ib)][:il, :])
                else:
                    s1 = ps_ty.tile([128, T], F32, tag="ty", name="ty")
                    nc.tensor.matmul(s1[:il, :], qaT[h][:, tsl],
                                     kaT[h][:, bsl], skip_group_check=True)
                    nc.vector.tensor_add(st[:il, :], s1[:il, :],
                                         mb[(b, ib)][:il, :])

                nmax = sm.tile([128, 1], F32, tag="nmax", name="nmax")
                nc.vector.tensor_reduce(nmax[:il], st[:il, :], AX.X, OP.max,
                                        negate=True)
                p = sm.tile([128, T], BF16, tag="p", name="p")
                rsum = sm.tile([128, 1], F32, tag="rsum", name="rsum")
                nc.scalar.activation(p[:il, :], st[:il, :], AF.Exp,
                                     bias=nmax[:il], accum_out=rsum[:il])
                rcp = sm.tile([128, 1], F32, tag="rcp", name="rcp")
                nc.vector.reciprocal(rcp[:il], rsum[:il])
                nc.vector.tensor_scalar(p[:il, :], p[:il, :], rcp[:il], None,
                                        OP.mult)

                hsl = slice(DK * h, DK * (h + 1))
                for jb, (j0, jl) in enumerate(IBLOCKS):
                    tp = ps_ty.tile([128, 128], BF16, tag="ty", name="ty")
                    nc.tensor.matmul(tp[:jl, :il], p[:il, j0 : j0 + jl],
                                     ident[:il, :il], is_transpose=True,
                                     skip_group_check=True)
                    pT = sm.tile([128, 128], BF16, tag="pT", name="pT")
                    nc.scalar.copy(pT[:jl, :il], tp[:jl, :il])
                    nc.tensor.matmul(xT_ps[hsl, :il], vb[(b, jb)][:jl, hsl],
                                     pT[:jl, :il],
                                     start=(jb == 0), stop=(jb == 1),
                                     skip_group_check=True,
                                     tile_position=(0, DK * h))

            xT_sb = sm.tile([128, 128], BF16, tag="xT_sb", name="xT_sb")
            nc.scalar.copy(xT_sb[:, :il], xT_ps[:, :il])
            y_ps = ps_ty.tile([128, 128], F32, tag="ty", name="ty")
            nc.tensor.matmul(y_ps[:il, :], xT_sb[:, :il], wo,
                             skip_group_check=True)
            y_sb = keep.tile([128, 128], F32, tag=f"y_out{b}_{ib}",
                             name=f"y_out{b}_{ib}")
            nc.vector.tensor_add(y_sb[:il, :], y_ps[:il, :], bo_b[:il, :])
            nc.scalar.dma_start(out[b, i0 : i0 + il, :], y_sb[:il, :])


def build_nc():
    nc = bacc.Bacc(trn_type="TRN2")
    io = {}
    io["inpack"] = nc.dram_tensor(
        "inpack", [128, IN_COLS], BF16, kind="ExternalInput").ap()
    # fp8 bytes carried as bf16: [h, b, d, i, j/2]
    io["rel_kernel"] = nc.dram_tensor(
        "rel_kernel", [H_REL, BPC, D, T, T // 2], BF16, kind="ExternalInput"
    ).ap()
    io["out"] = nc.dram_tensor("out", [BPC, T, D], F32,
                               kind="ExternalOutput").ap()

    with tile.TileContext(nc) as tc:
        with ExitStack() as ctx:
            build_kernel(ctx, tc, io)
    nc.compile()
    return nc


_NC_CACHE = None


def _get_nc():
    global _NC_CACHE
    if _NC_CACHE is None:
        _NC_CACHE = build_nc()
    return _NC_CACHE


def make_in_maps(inputs):
    """Shard full inputs into per-core input maps (layout/dtype work only)."""
    import ml_dtypes
    bf = ml_dtypes.bfloat16
    f32 = np.float32
    g = {k: np.asarray(inputs[k], dtype=f32) for k in
         ["Wq", "bq", "Wk", "bk", "Wv", "bv", "abs_q_w", "abs_q_b",
          "abs_k_w", "abs_k_b", "rel_k_w", "rel_k_b", "rel_bias",
          "Wo", "bo"]}
    wp = np.zeros((128, WPACK_COLS), f32)

    def put(nm, arr):
        o = WPACK_OFF[nm]
        arr = np.asarray(arr, f32)
        if arr.ndim == 1:
            arr = arr[:, None]
        wp[: arr.shape[0], o : o + arr.shape[1]] = arr

    put("Wq", g["Wq"]); put("Wk", g["Wk"]); put("Wv", g["Wv"])
    put("Wo", g["Wo"]); put("bq", g["bq"]); put("bk", g["bk"])
    for hr in range(H_REL):
        put(f"rkwT{hr}", g["rel_k_w"][hr].T)  # [32 o, 128 d]
        put(f"rkb{hr}", g["rel_k_b"][hr])
        put(f"rbias{hr}", g["rel_bias"][0, hr, 0, :])
    for hh in range(H_ABS):
        put(f"aqw{hh}", g["abs_q_w"][hh])
        put(f"akw{hh}", g["abs_k_w"][hh])
        put(f"aqb{hh}", g["abs_q_b"][hh])
        put(f"akb{hh}", g["abs_k_b"][hh])
    put("bvb", np.tile(g["bv"][None, :], (128, 1)))
    put("bob", np.tile(g["bo"][None, :], (128, 1)))

    query = np.asarray(inputs["query"], dtype=f32)
    key = np.asarray(inputs["key"], dtype=f32)
    value = np.asarray(inputs["value"], dtype=f32)
    mask_i = np.asarray(inputs["mask"], dtype=np.int32)[:, 0]  # [B, T, T]
    absk = np.asarray(inputs["abs_kernel"], dtype=f32)

    # rel_kernel: fp8 e4m3, host-transposed to [h, B, d, i, j]
    rk8 = np.asarray(inputs["rel_kernel"], dtype=f32).astype(
        ml_dtypes.float8_e4m3fn)                     # [h, B, i, j, d]
    rkT = np.ascontiguousarray(rk8.transpose(0, 1, 4, 2, 3))  # [h,B,d,i,j]
    relk = rkT.view(np.uint16).view(bf)              # [h, B, d, i, j/2]

    in_maps = []
    for c in range(N_CORES):
        bs = slice(c * BPC, (c + 1) * BPC)
        ip = np.zeros((128, IN_COLS), f32)
        ip[:, :WPACK_COLS] = wp
        ip[:, IN_OFF_Q : IN_OFF_Q + TT] = query[bs].reshape(TT, 128).T
        ip[:, IN_OFF_K : IN_OFF_K + TT] = key[bs].reshape(TT, 128).T
        ip[:, IN_OFF_V : IN_OFF_V + TT] = value[bs].reshape(TT, 128).T
        ip[:, IN_OFF_A0 : IN_OFF_A0 + TT] = absk[0, bs].reshape(TT, 128).T
        ip[:, IN_OFF_A1 : IN_OFF_A1 + TT] = absk[1, bs].reshape(TT, 128).T
        for bl in range(BPC):
            for ib, (i0, il) in enumerate(IBLOCKS):
                o = IN_OFF_M + (bl * 2 + ib) * T
                ip[:il, o : o + T] = mask_i[c * BPC + bl, i0 : i0 + il, :]
        m = {
            "inpack": np.ascontiguousarray(ip.astype(bf)),
            "rel_kernel": np.ascontiguousarray(relk[:, bs]),
        }
        in_maps.append(m)
    return in_maps


def kernel(**inputs) -> np.ndarray:
    nc = _get_nc()
    in_maps = make_in_maps(inputs)
    res = run_bass_kernel_spmd(nc, in_maps, core_ids=list(range(N_CORES)))
    return np.concatenate([r["out"] for r in res.results], axis=0)


if __name__ == "__main__":
    nc = build_nc()
    print("built ok")


# revision 13
# speedup vs baseline: 1.7319x; 1.0036x over previous
"""Trainium2 Bass kernel for nn_Attention_12463995093474 (sparse_attention).

Math (reference):
  q/k/v = content linears; 2 absolute heads, 2 relative heads (DK=32).
  abs:  scores = (Xq_a + abs_kernel@abs_q_w) @ (Xk_a + abs_kernel@abs_k_w)^T
  rel:  scores = Xq_r @ Xk_r^T + (Xq_r + rel_bias) . (rel_kernel@rel_k_w + rel_k_b)
  softmax(mask) @ v -> out linear.

Key algebraic collapse: the dominant rel term
    s2[i,j] = sum_o q''[i,o] * (sum_d RK[i,j,d] W[d,o])  (+ c[i])
            = sum_d qW[i,d] * RK[i,j,d],   qW = q'' @ W^T
so RK contracts DIRECTLY against a per-row vector (32x fewer MACs than
materializing R).  Implementation:
  - rel_kernel is pre-transposed on host to [h, b, d, i, j] fp8(e4m3)
    so it streams with PLAIN DMAs (no xbar transpose) at full HBM bw.
  - per (head h, row i) one PE matmul: lhsT = qW8 fp8 columns, rhs =
    RK[i]^T fp8 [128 x 200] -> 200 cols @ 1 cycle/row.  PSUM rows must
    land 32-aligned, so rows are processed DESCENDING within each
    32-row band: matmul for row r writes rows [band..band+r]
    (start=True); garbage in rows < r is overwritten by later matmuls,
    leaving every row's true matvec in place.
  - content scores accumulate into the same PSUM tile (start=False),
    then softmax -> p@v -> output linear, all matmuls in bf16.
  - qW is scaled by 64 on-device before the fp8 cast (values sigma
    ~0.005 would be subnormal in e4m3); the 2^-6 is folded into the
    epilogue's existing (s2 + c) * scale DVE op.

Sharding: data-parallel over batch, B=16 -> 2 batches per core on 8 cores.
"""

import numpy as np
from contextlib import ExitStack

import concourse.bass as bass
import concourse.bacc as bacc
import concourse.tile as tile
from concourse import mybir
from concourse.masks import make_identity
from concourse.bass_utils import run_bass_kernel_spmd

B, T, D = 16, 200, 128
H_ABS, H_REL, H, DK = 2, 2, 4, 32
N_CORES = 8
BPC = B // N_CORES  # batches per core
SCALE = 1.0 / float(DK) ** 0.5
TT = BPC * T  # tokens per core (400)
UP = 64.0     # qW fp8 upscale (power of 2)

F32 = mybir.dt.float32
BF16 = mybir.dt.bfloat16
F8 = mybir.dt.float8e4
AX = mybir.AxisListType
OP = mybir.AluOpType
AF = mybir.ActivationFunctionType

# i-blocks per batch: (start, len)
IBLOCKS = [(0, 128), (128, T - 128)]

# packed weights, stored [128, col] on host; loaded with one plain DMA
WPACK_LAYOUT = [
    ("Wq", 128, 128), ("Wk", 128, 128), ("Wv", 128, 128), ("Wo", 128, 128),
    ("bq", 128, 1), ("bk", 128, 1),
    ("rkwT0", 32, 128), ("rkwT1", 32, 128),
    ("rkb0", 32, 1), ("rkb1", 32, 1), ("rbias0", 32, 1), ("rbias1", 32, 1),
    ("aqw0", 128, 32), ("aqw1", 128, 32), ("akw0", 128, 32), ("akw1", 128, 32),
    ("aqb0", 32, 1), ("aqb1", 32, 1), ("akb0", 32, 1), ("akb1", 32, 1),
    ("bqrb0", 32, 1), ("bqrb1", 32, 1),
    ("bvb", 128, 128), ("bob", 128, 128),
]
WPACK_OFF = {}
_o = 0
for _nm, _r, _c in WPACK_LAYOUT:
    WPACK_OFF[_nm] = _o
    _o += _c
WPACK_COLS = _o
# one host-packed [128, IN_COLS] bf16 tensor: weights + q/k/v + absk + masks
IN_OFF_W = 0
IN_OFF_ID = WPACK_COLS
IN_OFF_Q = IN_OFF_ID + 128
IN_A_COLS = IN_OFF_Q + TT          # part A: weights + ident + xq
IN_OFF_K = IN_A_COLS
IN_OFF_V = IN_OFF_K + TT
IN_OFF_A0 = IN_OFF_V + TT
IN_OFF_A1 = IN_OFF_A0 + TT
IN_OFF_M = IN_OFF_A1 + TT
IN_COLS = IN_OFF_M + 4 * T

DEBUG = False


def chunks_for(il):
    """i-chunks (local_start, G) for the RK stream; 32-aligned bands."""
    out, i = [], 0
    while i < il:
        g = min(64, il - i)
        out.append((i, g))
        i += g
    return out


def build_kernel(ctx: ExitStack, tc: tile.TileContext, io: dict):
    nc = tc.nc

    relk = io["rel_kernel"]    # [2, 2, 128, 200, 100] bf16 carrier of fp8
    out = io["out"]            # [2, 200, 128]

    consts = ctx.enter_context(tc.tile_pool(name="consts", bufs=1))
    keep = ctx.enter_context(tc.tile_pool(name="keep", bufs=1))
    prep = ctx.enter_context(tc.tile_pool(name="prep", bufs=2))
    rkt_pool = ctx.enter_context(tc.tile_pool(name="rkt", bufs=5))
    sm = ctx.enter_context(tc.tile_pool(name="sm", bufs=2))
    ps_s2 = ctx.enter_context(tc.tile_pool(name="ps_s2", bufs=2, space="PSUM"))
    ps_pp = ctx.enter_context(tc.tile_pool(name="ps_pp", bufs=1, space="PSUM"))
    ps_x = ctx.enter_context(tc.tile_pool(name="ps_x", bufs=1, space="PSUM"))
    ps_ty = ctx.enter_context(tc.tile_pool(name="ps_ty", bufs=2, space="PSUM"))

    # ---------------- input load (plain DMAs only) ----------------
    # part A (weights+ident+xq) lands first so prep unblocks early;
    # both on the scalar queue so RK chunks own the gpsimd queue.
    inp = consts.tile([128, IN_COLS], BF16, tag="inpack")
    nc.scalar.dma_start(inp[:, :IN_A_COLS], io["inpack"][:, :IN_A_COLS])
    nc.scalar.dma_start(inp[:, IN_A_COLS:], io["inpack"][:, IN_A_COLS:])

    def wslice(nm, rows=128):
        o = WPACK_OFF[nm]
        c = dict((n, cc) for n, _r, cc in WPACK_LAYOUT)[nm]
        return inp[:rows, o : o + c]

    xqT = inp[:, IN_OFF_Q : IN_OFF_Q + TT]
    xkT = inp[:, IN_OFF_K : IN_OFF_K + TT]
    xvT = inp[:, IN_OFF_V : IN_OFF_V + TT]
    akT_pre = {0: inp[:, IN_OFF_A0 : IN_OFF_A0 + TT],
               1: inp[:, IN_OFF_A1 : IN_OFF_A1 + TT]}

    ident = inp[:, IN_OFF_ID : IN_OFF_ID + 128]

    # ---------------- prep ----------------
    wq = wslice("Wq")
    wk = wslice("Wk")
    wv = wslice("Wv")
    wo = wslice("Wo")

    bq_s = consts.tile([128, 1], F32, tag="bq_s")
    nc.scalar.activation(bq_s, wslice("bq"), AF.Copy, scale=SCALE)
    bq_s64 = consts.tile([128, 1], F32, tag="bq_s64")
    nc.scalar.activation(bq_s64, wslice("bq"), AF.Copy, scale=SCALE * UP)
    bk_c = consts.tile([128, 1], F32, tag="bk_c")
    nc.scalar.copy(bk_c, wslice("bk"))
    bv_b = consts.tile([128, 128], F32, tag="bv_b")
    nc.scalar.copy(bv_b, wslice("bvb"))
    bo_b = consts.tile([128, 128], F32, tag="bo_b")
    nc.scalar.copy(bo_b, wslice("bob"))

    small_cols = {}
    for hr in range(H_REL):
        ts_ = consts.tile([DK, 1], F32, tag=f"rbias_s64_{hr}")
        nc.scalar.activation(ts_, wslice(f"rbias{hr}", rows=DK), AF.Copy,
                             scale=SCALE * UP)
        small_cols[("rbias_s64", hr)] = ts_
    for hh in range(H_ABS):
        ts_ = consts.tile([DK, 1], F32, tag=f"aqb_s{hh}")
        nc.scalar.activation(ts_, wslice(f"aqb{hh}", rows=DK), AF.Copy,
                             scale=SCALE)
        small_cols[("aqb_s", hh)] = ts_
        akb32 = consts.tile([DK, 1], F32, tag=f"akb32_{hh}")
        nc.scalar.copy(akb32, wslice(f"akb{hh}", rows=DK))
        small_cols[("akb", hh)] = akb32

    # combined bias cols (bq + rbias)*SCALE*64, packed on host
    bqrb64 = {}
    for hr in range(H_REL):
        t = consts.tile([DK, 1], F32, tag=f"bqrb64_{hr}")
        nc.scalar.activation(t, wslice(f"bqrb{hr}", rows=DK), AF.Copy,
                             scale=SCALE * UP)
        bqrb64[hr] = t

    # content q projections: abs heads scaled by SCALE, rel heads by SCALE*64
    qT = {}      # abs heads, bf16 [32, TT]
    qT64 = {}    # rel heads (x64), bf16 [32, TT]
    qrb64 = {}   # rel heads with rel_bias folded in, bf16 [32, TT]
    qW8 = {}     # fp8 [128 d, TT]
    for h in (2, 3, 0, 1):  # rel heads first: they gate the matvec stream
        qp = ps_pp.tile([DK, TT], F32, tag="pp", name="pp")
        nc.tensor.matmul(qp, wq[:, DK * h : DK * (h + 1)], xqT,
                         skip_group_check=True)
        t = keep.tile([DK, TT], BF16, tag=f"qT{h}", name=f"qT{h}")
        if h < H_ABS:
            nc.scalar.activation(t, qp, AF.Identity,
                                 bias=bq_s[DK * h : DK * (h + 1)], scale=SCALE)
            qT[h] = t
        else:
            hr = h - H_ABS
            t2 = keep.tile([DK, TT], BF16, tag=f"qrb64_{hr}",
                           name=f"qrb64_{hr}")
            nc.scalar.activation(t2, qp, AF.Identity, bias=bqrb64[hr],
                                 scale=SCALE * UP)
            qrb64[hr] = t2
            qwp = ps_pp.tile([128, TT], F32, tag="pp", name="pp")
            nc.tensor.matmul(qwp, wslice(f"rkwT{hr}", rows=DK), t2,
                             skip_group_check=True)
            q8 = keep.tile([128, TT], F8, tag=f"qW8_{hr}", name=f"qW8_{hr}")
            nc.vector.tensor_copy(q8, qwp)
            qW8[hr] = q8
            nc.scalar.activation(t, qp, AF.Identity,
                                 bias=bq_s64[DK * h : DK * (h + 1)],
                                 scale=SCALE * UP)
            qT64[hr] = t

    # content k projections (raw) bf16 [32, TT] per head
    kT = {}
    for h in range(H):
        kp = ps_pp.tile([DK, TT], F32, tag="pp", name="pp")
        nc.tensor.matmul(kp, wk[:, DK * h : DK * (h + 1)], xkT,
                         skip_group_check=True)
        t = keep.tile([DK, TT], BF16, tag=f"kT{h}", name=f"kT{h}")
        nc.scalar.activation(t, kp, AF.Identity,
                             bias=bk_c[DK * h : DK * (h + 1)])
        kT[h] = t

    # values, bf16 [jl, 128] per (b, jblock)
    vb = {}
    for b in range(BPC):
        for jb, (j0, jl) in enumerate(IBLOCKS):
            vp = ps_ty.tile([128, 128], F32, tag="ty", name="ty")
            nc.tensor.matmul(vp[:jl, :],
                             xvT[:, b * T + j0 : b * T + j0 + jl], wv,
                             skip_group_check=True)
            t = keep.tile([128, 128], BF16, tag=f"v{b}_{jb}",
                          name=f"v{b}_{jb}")
            nc.vector.tensor_add(t[:jl, :], vp[:jl, :], bv_b[:jl, :])
            vb[(b, jb)] = t

    # absolute-position heads: qaT = qT + Pq, kaT = kT + Pk (bf16 [32, TT])
    qaT = {}
    kaT = {}
    for hh in range(H_ABS):
        akT = akT_pre[hh]
        pp = ps_pp.tile([DK, TT], F32, tag="pp", name="pp")
        nc.tensor.matmul(pp, wslice(f"aqw{hh}"), akT, skip_group_check=True)
        pqT = prep.tile([DK, TT], BF16, tag="pqT", name="pqT")
        nc.scalar.activation(pqT, pp, AF.Identity,
                             bias=small_cols[("aqb_s", hh)], scale=SCALE)
        t = keep.tile([DK, TT], BF16, tag=f"qaT{hh}", name=f"qaT{hh}")
        nc.vector.tensor_add(t, qT[hh], pqT)
        qaT[hh] = t

        pp2 = ps_pp.tile([DK, TT], F32, tag="pp", name="pp")
        nc.tensor.matmul(pp2, wslice(f"akw{hh}"), akT, skip_group_check=True)
        pkT = prep.tile([DK, TT], BF16, tag="pqT", name="pqT")
        nc.scalar.activation(pkT, pp2, AF.Identity,
                             bias=small_cols[("akb", hh)])
        t = keep.tile([DK, TT], BF16, tag=f"kaT{hh}", name=f"kaT{hh}")
        nc.vector.tensor_add(t, kT[hh], pkT)
        kaT[hh] = t

    # masks: mb = mask*1e9 - 1e9, f32 [il, T]
    mb = {}
    for b in range(BPC):
        for ib, (i0, il) in enumerate(IBLOCKS):
            o = IN_OFF_M + (b * 2 + ib) * T
            t = keep.tile([128, T], F32, tag=f"mb{b}_{ib}", name=f"mb{b}_{ib}")
            nc.vector.tensor_scalar(t[:il, :], inp[:il, o : o + T], 1e9, -1e9,
                                    OP.mult, OP.add)
            mb[(b, ib)] = t

    # c64[i] = qrb64 . rkb  (carries the x64; f32 col per (hr, b, ib))
    c_sb = {}
    for hr in range(H_REL):
        for b in range(BPC):
            for ib, (i0, il) in enumerate(IBLOCKS):
                tsl = slice(b * T + i0, b * T + i0 + il)
                cp = ps_ty.tile([128, 1], F32, tag="ty", name="ty")
                nc.tensor.matmul(cp[:il, :], qrb64[hr][:, tsl],
                                 wslice(f"rkb{hr}", rows=DK),
                                 skip_group_check=True)
                t = keep.tile([128, 1], F32, tag=f"c{hr}_{b}_{ib}",
                              name=f"c{hr}_{b}_{ib}")
                nc.scalar.copy(t[:il, :], cp[:il, :])
                c_sb[(hr, b, ib)] = t

    # ---------------- main loop ----------------
    for b in range(BPC):
        for ib, (i0, il) in enumerate(IBLOCKS):
            bsl = slice(b * T, (b + 1) * T)
            tsl = slice(b * T + i0, b * T + i0 + il)
            last = (b == BPC - 1 and ib == len(IBLOCKS) - 1)
            xT_ps = ps_x.tile([128, 128], F32, tag="xT", name="xT")

            def emit_head(h, s2ps):
                is_rel = h >= H_ABS
                st = sm.tile([128, T], F32, tag="st", name="st")
                if is_rel:
                    hr = h - H_ABS
                    # st = (s2 + c64) * 2^-6 + mask
                    nc.vector.tensor_scalar(st[:il, :], s2ps[hr][:il, :T],
                                            c_sb[(hr, b, ib)][:il],
                                            1.0 / UP, OP.add, OP.mult)
                    nc.vector.tensor_add(st[:il, :], st[:il, :],
                                         mb[(b, ib)][:il, :])
                else:
                    s1 = ps_ty.tile([128, T], F32, tag="ty", name="ty")
                    nc.tensor.matmul(s1[:il, :], qaT[h][:, tsl],
                                     kaT[h][:, bsl], skip_group_check=True)
                    nc.vector.tensor_add(st[:il, :], s1[:il, :],
                                         mb[(b, ib)][:il, :])

                nmax = sm.tile([128, 1], F32, tag="nmax", name="nmax")
                nc.vector.tensor_reduce(nmax[:il], st[:il, :], AX.X, OP.max,
                                        negate=True)
                p = sm.tile([128, T], BF16, tag="p", name="p")
                rsum = sm.tile([128, 1], F32, tag="rsum", name="rsum")
                nc.scalar.activation(p[:il, :], st[:il, :], AF.Exp,
                                     bias=nmax[:il], accum_out=rsum[:il])
                rcp = sm.tile([128, 1], F32, tag="rcp", name="rcp")
                nc.vector.reciprocal(rcp[:il], rsum[:il])
                nc.vector.tensor_scalar(p[:il, :], p[:il, :], rcp[:il], None,
                                        OP.mult)

                hsl = slice(DK * h, DK * (h + 1))
                for jb, (j0, jl) in enumerate(IBLOCKS):
                    tp = ps_ty.tile([128, 128], BF16, tag="ty", name="ty")
                    nc.tensor.matmul(tp[:jl, :il], p[:il, j0 : j0 + jl],
                                     ident[:il, :il], is_transpose=True,
                                     skip_group_check=True)
                    pT = sm.tile([128, 128], BF16, tag="pT", name="pT")
                    nc.scalar.copy(pT[:jl, :il], tp[:jl, :il])
                    nc.tensor.matmul(xT_ps[hsl, :il], vb[(b, jb)][:jl, hsl],
                                     pT[:jl, :il],
                                     start=(jb == 0), stop=(jb == 1),
                                     skip_group_check=True,
                                     tile_position=(0, DK * h))

            if last:
                # abs heads depend only on prep: run them under the rel
                # streams so only the rel chain remains at the end
                emit_head(0, None)
                emit_head(1, None)

            s2ps = {}
            for hr in range(H_REL):
                # full-bank rows (512 f32 = 2048B) so per-row matmul writes
                # land bank-aligned; only [:, :T] is used
                s2t = ps_s2.tile([128, 512], F32, tag=f"s2h{hr}",
                                 name=f"s2h{hr}")
                s2ps[hr] = s2t
                # stream RK chunks + per-row matvec matmuls
                for (ic0, G) in chunks_for(il):
                    rkt = rkt_pool.tile([128, 6400], BF16, tag="rkt",
                                        name="rkt")
                    nc.gpsimd.dma_start(
                        rkt[:, : G * 100],
                        relk[hr, b][:, i0 + ic0 : i0 + ic0 + G, :])
                    rkt8 = rkt.bitcast(F8)
                    # 32-row bands; descending rows within each band: row
                    # r's matmul writes rows [0..r] of the band, start=True
                    # zeroes/overwrites garbage below it.
                    for b0 in range(0, G, 32):
                        gl = min(32, G - b0)
                        for r in range(gl - 1, -1, -1):
                            gb = b * T + i0 + ic0 + b0
                            nc.tensor.matmul(
                                s2t[ic0 + b0 : ic0 + b0 + r + 1, :T],
                                qW8[hr][:, gb : gb + r + 1],
                                rkt8[:, (b0 + r) * T : (b0 + r + 1) * T],
                                start=True, stop=False,
                                skip_group_check=True,
                                tile_position=(0, ic0 + b0))
                # content scores accumulate on top
                nc.tensor.matmul(s2t[:il, :T], qT64[hr][:, tsl],
                                 kT[H_ABS + hr][:, bsl],
                                 start=False, stop=True,
                                 skip_group_check=True)

            for h in ((2, 3) if last else (2, 3, 0, 1)):
                emit_head(h, s2ps)

            xT_sb = sm.tile([128, 128], BF16, tag="xT_sb", name="xT_sb")
            nc.scalar.copy(xT_sb[:, :il], xT_ps[:, :il])
            y_ps = ps_ty.tile([128, 128], F32, tag="ty", name="ty")
            nc.tensor.matmul(y_ps[:il, :], xT_sb[:, :il], wo,
                             skip_group_check=True)
            y_sb = keep.tile([128, 128], F32, tag=f"y_out{b}_{ib}",
                             name=f"y_out{b}_{ib}")
            nc.vector.tensor_add(y_sb[:il, :], y_ps[:il, :], bo_b[:il, :])
            nc.scalar.dma_start(out[b, i0 : i0 + il, :], y_sb[:il, :])


def build_nc():
    nc = bacc.Bacc(trn_type="TRN2")
    io = {}
    io["inpack"] = nc.dram_tensor(
        "inpack", [128, IN_COLS], BF16, kind="ExternalInput").ap()
    # fp8 bytes carried as bf16: [h, b, d, i, j/2]
    io["rel_kernel"] = nc.dram_tensor(
        "rel_kernel", [H_REL, BPC, D, T, T // 2], BF16, kind="ExternalInput"
    ).ap()
    io["out"] = nc.dram_tensor("out", [BPC, T, D], F32,
                               kind="ExternalOutput").ap()

    with tile.TileContext(nc) as tc:
        with ExitStack() as ctx:
            build_kernel(ctx, tc, io)
    nc.compile()
    return nc


_NC_CACHE = None


def _get_nc():
    global _NC_CACHE
    if _NC_CACHE is None:
        _NC_CACHE = build_nc()
    return _NC_CACHE


def make_in_maps(inputs):
    """Shard full inputs into per-core input maps (layout/dtype work only)."""
    import ml_dtypes
    bf = ml_dtypes.bfloat16
    f32 = np.float32
    g = {k: np.asarray(inputs[k], dtype=f32) for k in
         ["Wq", "bq", "Wk", "bk", "Wv", "bv", "abs_q_w", "abs_q_b",
          "abs_k_w", "abs_k_b", "rel_k_w", "rel_k_b", "rel_bias",
          "Wo", "bo"]}
    wp = np.zeros((128, WPACK_COLS), f32)

    def put(nm, arr):
        o = WPACK_OFF[nm]
        arr = np.asarray(arr, f32)
        if arr.ndim == 1:
            arr = arr[:, None]
        wp[: arr.shape[0], o : o + arr.shape[1]] = arr

    put("Wq", g["Wq"]); put("Wk", g["Wk"]); put("Wv", g["Wv"])
    put("Wo", g["Wo"]); put("bq", g["bq"]); put("bk", g["bk"])
    for hr in range(H_REL):
        put(f"rkwT{hr}", g["rel_k_w"][hr].T)  # [32 o, 128 d]
        put(f"rkb{hr}", g["rel_k_b"][hr])
        put(f"rbias{hr}", g["rel_bias"][0, hr, 0, :])
        put(f"bqrb{hr}", g["bq"][DK * (H_ABS + hr) : DK * (H_ABS + hr + 1)]
            + g["rel_bias"][0, hr, 0, :])
    for hh in range(H_ABS):
        put(f"aqw{hh}", g["abs_q_w"][hh])
        put(f"akw{hh}", g["abs_k_w"][hh])
        put(f"aqb{hh}", g["abs_q_b"][hh])
        put(f"akb{hh}", g["abs_k_b"][hh])
    put("bvb", np.tile(g["bv"][None, :], (128, 1)))
    put("bob", np.tile(g["bo"][None, :], (128, 1)))

    query = np.asarray(inputs["query"], dtype=f32)
    key = np.asarray(inputs["key"], dtype=f32)
    value = np.asarray(inputs["value"], dtype=f32)
    mask_i = np.asarray(inputs["mask"], dtype=np.int32)[:, 0]  # [B, T, T]
    absk = np.asarray(inputs["abs_kernel"], dtype=f32)

    # rel_kernel: fp8 e4m3, host-transposed to [h, B, d, i, j]
    rk8 = np.asarray(inputs["rel_kernel"], dtype=f32).astype(
        ml_dtypes.float8_e4m3fn)                     # [h, B, i, j, d]
    rkT = np.ascontiguousarray(rk8.transpose(0, 1, 4, 2, 3))  # [h,B,d,i,j]
    relk = rkT.view(np.uint16).view(bf)              # [h, B, d, i, j/2]

    in_maps = []
    for c in range(N_CORES):
        bs = slice(c * BPC, (c + 1) * BPC)
        ip = np.zeros((128, IN_COLS), f32)
        ip[:, :WPACK_COLS] = wp
        ip[:, IN_OFF_ID : IN_OFF_ID + 128] = np.eye(128, dtype=f32)
        ip[:, IN_OFF_Q : IN_OFF_Q + TT] = query[bs].reshape(TT, 128).T
        ip[:, IN_OFF_K : IN_OFF_K + TT] = key[bs].reshape(TT, 128).T
        ip[:, IN_OFF_V : IN_OFF_V + TT] = value[bs].reshape(TT, 128).T
        ip[:, IN_OFF_A0 : IN_OFF_A0 + TT] = absk[0, bs].reshape(TT, 128).T
        ip[:, IN_OFF_A1 : IN_OFF_A1 + TT] = absk[1, bs].reshape(TT, 128).T
        for bl in range(BPC):
            for ib, (i0, il) in enumerate(IBLOCKS):
                o = IN_OFF_M + (bl * 2 + ib) * T
                ip[:il, o : o + T] = mask_i[c * BPC + bl, i0 : i0 + il, :]
        m = {
            "inpack": np.ascontiguousarray(ip.astype(bf)),
            "rel_kernel": np.ascontiguousarray(relk[:, bs]),
        }
        in_maps.append(m)
    return in_maps


def kernel(**inputs) -> np.ndarray:
    nc = _get_nc()
    in_maps = make_in_maps(inputs)
    res = run_bass_kernel_spmd(nc, in_maps, core_ids=list(range(N_CORES)))
    return np.concatenate([r["out"] for r in res.results], axis=0)


if __name__ == "__main__":
    nc = build_nc()
    print("built ok")


# revision 16
# speedup vs baseline: 1.8454x; 1.0655x over previous
"""Trainium2 Bass kernel for nn_Attention_12463995093474 (sparse_attention).

Math (reference):
  q/k/v = content linears; 2 absolute heads, 2 relative heads (DK=32).
  abs:  scores = (Xq_a + abs_kernel@abs_q_w) @ (Xk_a + abs_kernel@abs_k_w)^T
  rel:  scores = Xq_r @ Xk_r^T + (Xq_r + rel_bias) . (rel_kernel@rel_k_w + rel_k_b)
  softmax(mask) @ v -> out linear.

Key algebraic collapse: the dominant rel term
    s2[i,j] = sum_o q''[i,o] * (sum_d RK[i,j,d] W[d,o])  (+ c[i])
            = sum_d qW[i,d] * RK[i,j,d],   qW = q'' @ W^T
so RK contracts DIRECTLY against a per-row vector (32x fewer MACs than
materializing R).  Implementation:
  - rel_kernel is pre-transposed on host to [h, b, d, i, j] fp8(e4m3)
    so it streams with PLAIN DMAs (no xbar transpose) at full HBM bw.
  - per (head h, row i) one PE matmul: lhsT = qW8 fp8 columns, rhs =
    RK[i]^T fp8 [128 x 200] -> 200 cols @ 1 cycle/row.  PSUM rows must
    land 32-aligned, so rows are processed DESCENDING within each
    32-row band: matmul for row r writes rows [band..band+r]
    (start=True); garbage in rows < r is overwritten by later matmuls,
    leaving every row's true matvec in place.
  - content scores accumulate into the same PSUM tile (start=False),
    then softmax -> p@v -> output linear, all matmuls in bf16.
  - qW is scaled by 64 on-device before the fp8 cast (values sigma
    ~0.005 would be subnormal in e4m3); the 2^-6 is folded into the
    epilogue's existing (s2 + c) * scale DVE op.

Sharding: data-parallel over batch, B=16 -> 2 batches per core on 8 cores.
"""

import numpy as np
from contextlib import ExitStack

import concourse.bass as bass
import concourse.bacc as bacc
import concourse.tile as tile
from concourse import mybir
from concourse.masks import make_identity
from concourse.bass_utils import run_bass_kernel_spmd

B, T, D = 16, 200, 128
H_ABS, H_REL, H, DK = 2, 2, 4, 32
N_CORES = 8
BPC = B // N_CORES  # batches per core
SCALE = 1.0 / float(DK) ** 0.5
TT = BPC * T  # tokens per core (400)
UP = 64.0     # qW fp8 upscale (power of 2)

F32 = mybir.dt.float32
BF16 = mybir.dt.bfloat16
F8 = mybir.dt.float8e4
AX = mybir.AxisListType
OP = mybir.AluOpType
AF = mybir.ActivationFunctionType

# i-blocks per batch: (start, len)
IBLOCKS = [(0, 128), (128, T - 128)]

# packed weights, stored [128, col] on host; loaded with one plain DMA
WPACK_LAYOUT = [
    ("Wq", 128, 128), ("Wk", 128, 128), ("Wv", 128, 128), ("Wo", 128, 128),
    ("bq", 128, 1), ("bk", 128, 1),
    ("rkwT0", 32, 128), ("rkwT1", 32, 128),
    ("rkb0", 32, 1), ("rkb1", 32, 1), ("rbias0", 32, 1), ("rbias1", 32, 1),
    ("aqw0", 128, 32), ("aqw1", 128, 32), ("akw0", 128, 32), ("akw1", 128, 32),
    ("aqb0", 32, 1), ("aqb1", 32, 1), ("akb0", 32, 1), ("akb1", 32, 1),
    ("bqrb0", 32, 1), ("bqrb1", 32, 1),
    ("bvb", 128, 128), ("bob", 128, 128),
]
WPACK_OFF = {}
_o = 0
for _nm, _r, _c in WPACK_LAYOUT:
    WPACK_OFF[_nm] = _o
    _o += _c
WPACK_COLS = _o
# one host-packed [128, IN_COLS] bf16 tensor: weights + q/k/v + absk + masks
IN_OFF_W = 0
IN_OFF_ID = WPACK_COLS
IN_OFF_Q = IN_OFF_ID + 128
IN_A_COLS = IN_OFF_Q + TT          # part A: weights + ident + xq
IN_OFF_K = IN_A_COLS
IN_OFF_V = IN_OFF_K + TT
IN_OFF_A0 = IN_OFF_V + TT
IN_OFF_A1 = IN_OFF_A0 + TT
IN_OFF_M = IN_OFF_A1 + TT
IN_COLS = IN_OFF_M + 4 * T

DEBUG = False


def chunks_for(il):
    """i-chunks (local_start, G) for the RK stream; 32-aligned bands."""
    out, i = [], 0
    while i < il:
        g = min(64, il - i)
        out.append((i, g))
        i += g
    return out


def build_kernel(ctx: ExitStack, tc: tile.TileContext, io: dict):
    nc = tc.nc

    relk = io["rel_kernel"]    # [2, 2, 128, 200, 100] bf16 carrier of fp8
    out = io["out"]            # [2, 200, 128]

    consts = ctx.enter_context(tc.tile_pool(name="consts", bufs=1))
    keep = ctx.enter_context(tc.tile_pool(name="keep", bufs=1))
    prep = ctx.enter_context(tc.tile_pool(name="prep", bufs=2))
    rkt_pool = ctx.enter_context(tc.tile_pool(name="rkt", bufs=5))
    sm = ctx.enter_context(tc.tile_pool(name="sm", bufs=2))
    ps_s2 = ctx.enter_context(tc.tile_pool(name="ps_s2", bufs=2, space="PSUM"))
    ps_pp = ctx.enter_context(tc.tile_pool(name="ps_pp", bufs=1, space="PSUM"))
    ps_x = ctx.enter_context(tc.tile_pool(name="ps_x", bufs=1, space="PSUM"))
    ps_ty = ctx.enter_context(tc.tile_pool(name="ps_ty", bufs=2, space="PSUM"))

    # ---------------- input load (plain DMAs only) ----------------
    # part A (weights+ident+xq) lands first so prep unblocks early;
    # both on the scalar queue so RK chunks own the gpsimd queue.
    inp = consts.tile([128, IN_COLS], BF16, tag="inpack")
    nc.sync.dma_start(inp[:, :IN_A_COLS], io["inpack"][:, :IN_A_COLS])
    nc.scalar.dma_start(inp[:, IN_A_COLS:], io["inpack"][:, IN_A_COLS:])

    def wslice(nm, rows=128):
        o = WPACK_OFF[nm]
        c = dict((n, cc) for n, _r, cc in WPACK_LAYOUT)[nm]
        return inp[:rows, o : o + c]

    xqT = inp[:, IN_OFF_Q : IN_OFF_Q + TT]
    xkT = inp[:, IN_OFF_K : IN_OFF_K + TT]
    xvT = inp[:, IN_OFF_V : IN_OFF_V + TT]
    akT_pre = {0: inp[:, IN_OFF_A0 : IN_OFF_A0 + TT],
               1: inp[:, IN_OFF_A1 : IN_OFF_A1 + TT]}

    ident = inp[:, IN_OFF_ID : IN_OFF_ID + 128]

    # ---------------- prep ----------------
    wq = wslice("Wq")
    wk = wslice("Wk")
    wv = wslice("Wv")
    wo = wslice("Wo")

    # ---- early consts: only what gates the matvec streams ----
    # combined bias cols (bq + rbias)*SCALE*64, packed on host
    bqrb64 = {}
    for hr in range(H_REL):
        t = consts.tile([DK, 1], F32, tag=f"bqrb64_{hr}")
        nc.scalar.activation(t, wslice(f"bqrb{hr}", rows=DK), AF.Copy,
                             scale=SCALE * UP)
        bqrb64[hr] = t

    qrb64 = {}   # rel heads q + rel_bias (x64), bf16 [32, TT]
    qW8 = {}     # fp8 [128 d, TT]

    def emit_rel_qw(hr):
        h = H_ABS + hr
        qp = ps_pp.tile([DK, TT], F32, tag="pp", name="pp")
        nc.tensor.matmul(qp, wq[:, DK * h : DK * (h + 1)], xqT,
                         skip_group_check=True)
        t2 = keep.tile([DK, TT], BF16, tag=f"qrb64_{hr}", name=f"qrb64_{hr}")
        nc.scalar.activation(t2, qp, AF.Identity, bias=bqrb64[hr],
                             scale=SCALE * UP)
        qrb64[hr] = t2
        qwp = ps_pp.tile([128, TT], F32, tag="pp", name="pp")
        nc.tensor.matmul(qwp, wslice(f"rkwT{hr}", rows=DK), t2,
                         skip_group_check=True)
        q8 = keep.tile([128, TT], F8, tag=f"qW8_{hr}", name=f"qW8_{hr}")
        nc.vector.tensor_copy(q8, qwp)
        qW8[hr] = q8

    # ---- deferred prep (emitted under the first matvec streams) ----
    qT = {}
    qT64 = {}
    kT = {}
    vb = {}
    qaT = {}
    kaT = {}
    mb = {}
    c_sb = {}

    def emit_rest_prep():
        bq_s = consts.tile([128, 1], F32, tag="bq_s")
        nc.scalar.activation(bq_s, wslice("bq"), AF.Copy, scale=SCALE)
        bq_s64 = consts.tile([128, 1], F32, tag="bq_s64")
        nc.scalar.activation(bq_s64, wslice("bq"), AF.Copy, scale=SCALE * UP)
        bk_c = consts.tile([128, 1], F32, tag="bk_c")
        nc.scalar.copy(bk_c, wslice("bk"))
        bv_b = consts.tile([128, 128], F32, tag="bv_b")
        nc.scalar.copy(bv_b, wslice("bvb"))
        bo_b = consts.tile([128, 128], F32, tag="bo_b")
        nc.scalar.copy(bo_b, wslice("bob"))
        reph["bo_b"] = bo_b

        small_cols = {}
        for hh in range(H_ABS):
            ts_ = consts.tile([DK, 1], F32, tag=f"aqb_s{hh}")
            nc.scalar.activation(ts_, wslice(f"aqb{hh}", rows=DK), AF.Copy,
                                 scale=SCALE)
            small_cols[("aqb_s", hh)] = ts_
            akb32 = consts.tile([DK, 1], F32, tag=f"akb32_{hh}")
            nc.scalar.copy(akb32, wslice(f"akb{hh}", rows=DK))
            small_cols[("akb", hh)] = akb32

        # content q for abs heads (SCALE) and qT64 for rel content (x64)
        for h in range(H):
            qp = ps_pp.tile([DK, TT], F32, tag="pp", name="pp")
            nc.tensor.matmul(qp, wq[:, DK * h : DK * (h + 1)], xqT,
                             skip_group_check=True)
            t = keep.tile([DK, TT], BF16, tag=f"qT{h}", name=f"qT{h}")
            if h < H_ABS:
                nc.scalar.activation(t, qp, AF.Identity,
                                     bias=bq_s[DK * h : DK * (h + 1)],
                                     scale=SCALE)
                qT[h] = t
            else:
                nc.scalar.activation(t, qp, AF.Identity,
                                     bias=bq_s64[DK * h : DK * (h + 1)],
                                     scale=SCALE * UP)
                qT64[h - H_ABS] = t

        for h in range(H):
            kp = ps_pp.tile([DK, TT], F32, tag="pp", name="pp")
            nc.tensor.matmul(kp, wk[:, DK * h : DK * (h + 1)], xkT,
                             skip_group_check=True)
            t = keep.tile([DK, TT], BF16, tag=f"kT{h}", name=f"kT{h}")
            nc.scalar.activation(t, kp, AF.Identity,
                                 bias=bk_c[DK * h : DK * (h + 1)])
            kT[h] = t

        for b in range(BPC):
            for jb, (j0, jl) in enumerate(IBLOCKS):
                vp = ps_ty.tile([128, 128], F32, tag="ty", name="ty")
                nc.tensor.matmul(vp[:jl, :],
                                 xvT[:, b * T + j0 : b * T + j0 + jl], wv,
                                 skip_group_check=True)
                t = keep.tile([128, 128], BF16, tag=f"v{b}_{jb}",
                              name=f"v{b}_{jb}")
                nc.vector.tensor_add(t[:jl, :], vp[:jl, :], bv_b[:jl, :])
                vb[(b, jb)] = t

        for hh in range(H_ABS):
            akT = akT_pre[hh]
            pp = ps_pp.tile([DK, TT], F32, tag="pp", name="pp")
            nc.tensor.matmul(pp, wslice(f"aqw{hh}"), akT,
                             skip_group_check=True)
            pqT = prep.tile([DK, TT], BF16, tag="pqT", name="pqT")
            nc.scalar.activation(pqT, pp, AF.Identity,
                                 bias=small_cols[("aqb_s", hh)], scale=SCALE)
            t = keep.tile([DK, TT], BF16, tag=f"qaT{hh}", name=f"qaT{hh}")
            nc.vector.tensor_add(t, qT[hh], pqT)
            qaT[hh] = t

            pp2 = ps_pp.tile([DK, TT], F32, tag="pp", name="pp")
            nc.tensor.matmul(pp2, wslice(f"akw{hh}"), akT,
                             skip_group_check=True)
            pkT = prep.tile([DK, TT], BF16, tag="pqT", name="pqT")
            nc.scalar.activation(pkT, pp2, AF.Identity,
                                 bias=small_cols[("akb", hh)])
            t = keep.tile([DK, TT], BF16, tag=f"kaT{hh}", name=f"kaT{hh}")
            nc.vector.tensor_add(t, kT[hh], pkT)
            kaT[hh] = t

        for b in range(BPC):
            for ib, (i0, il) in enumerate(IBLOCKS):
                o = IN_OFF_M + (b * 2 + ib) * T
                t = keep.tile([128, T], F32, tag=f"mb{b}_{ib}",
                              name=f"mb{b}_{ib}")
                nc.vector.tensor_scalar(t[:il, :], inp[:il, o : o + T],
                                        1e9, -1e9, OP.mult, OP.add)
                mb[(b, ib)] = t

        for hr in range(H_REL):
            for b in range(BPC):
                for ib, (i0, il) in enumerate(IBLOCKS):
                    tsl = slice(b * T + i0, b * T + i0 + il)
                    cp = ps_ty.tile([128, 1], F32, tag="ty", name="ty")
                    nc.tensor.matmul(cp[:il, :], qrb64[hr][:, tsl],
                                     wslice(f"rkb{hr}", rows=DK),
                                     skip_group_check=True)
                    t = keep.tile([128, 1], F32, tag=f"c{hr}_{b}_{ib}",
                                  name=f"c{hr}_{b}_{ib}")
                    nc.scalar.copy(t[:il, :], cp[:il, :])
                    c_sb[(hr, b, ib)] = t

    reph = {}

    # ---------------- stream / epilogue machinery ----------------
    def emit_stream(b, ib, hr, s2t, chunks):
        i0, il = IBLOCKS[ib]
        for (ic0, G) in chunks:
            rkt = rkt_pool.tile([128, 6400], BF16, tag="rkt", name="rkt")
            nc.gpsimd.dma_start(
                rkt[:, : G * 100],
                relk[hr, b][:, i0 + ic0 : i0 + ic0 + G, :])
            rkt8 = rkt.bitcast(F8)
            # 32-row bands; descending rows within each band: row r's
            # matmul writes rows [0..r] of the band, start=True
            # zeroes/overwrites garbage below it.
            for b0 in range(0, G, 32):
                gl = min(32, G - b0)
                for r in range(gl - 1, -1, -1):
                    gb = b * T + i0 + ic0 + b0
                    nc.tensor.matmul(
                        s2t[ic0 + b0 : ic0 + b0 + r + 1, :T],
                        qW8[hr][:, gb : gb + r + 1],
                        rkt8[:, (b0 + r) * T : (b0 + r + 1) * T],
                        start=True, stop=False,
                        skip_group_check=True,
                        tile_position=(0, ic0 + b0))

    def emit_content(b, ib, hr, s2t):
        i0, il = IBLOCKS[ib]
        nc.tensor.matmul(s2t[:il, :T],
                         qT64[hr][:, b * T + i0 : b * T + i0 + il],
                         kT[H_ABS + hr][:, b * T : (b + 1) * T],
                         start=False, stop=True, skip_group_check=True)

    def emit_head_scores(b, ib, h, s2ps):
        """DVE/ACT part: scores -> masked softmax -> p (bf16)."""
        i0, il = IBLOCKS[ib]
        tsl = slice(b * T + i0, b * T + i0 + il)
        bsl = slice(b * T, (b + 1) * T)
        st = sm.tile([128, T], F32, tag="st", name="st")
        if h >= H_ABS:
            hr = h - H_ABS
            # st = (s2 + c64) * 2^-6 + mask
            nc.vector.tensor_scalar(st[:il, :], s2ps[hr][:il, :T],
                                    c_sb[(hr, b, ib)][:il],
                                    1.0 / UP, OP.add, OP.mult)
            nc.vector.tensor_add(st[:il, :], st[:il, :],
                                 mb[(b, ib)][:il, :])
        else:
            s1 = ps_ty.tile([128, T], F32, tag="ty", name="ty")
            nc.tensor.matmul(s1[:il, :], qaT[h][:, tsl],
                             kaT[h][:, bsl], skip_group_check=True)
            nc.vector.tensor_add(st[:il, :], s1[:il, :],
                                 mb[(b, ib)][:il, :])
        nmax = sm.tile([128, 1], F32, tag="nmax", name="nmax")
        nc.vector.tensor_reduce(nmax[:il], st[:il, :], AX.X, OP.max,
                                negate=True)
        p = sm.tile([128, T], BF16, tag="p", name="p")
        rsum = sm.tile([128, 1], F32, tag="rsum", name="rsum")
        nc.scalar.activation(p[:il, :], st[:il, :], AF.Exp,
                             bias=nmax[:il], accum_out=rsum[:il])
        rcp = sm.tile([128, 1], F32, tag="rcp", name="rcp")
        nc.vector.reciprocal(rcp[:il], rsum[:il])
        nc.vector.tensor_scalar(p[:il, :], p[:il, :], rcp[:il], None,
                                OP.mult)
        return p

    def emit_head_pv(b, ib, h, p, xT_ps):
        """PE part: p transpose + p@v accumulation into xT_ps."""
        i0, il = IBLOCKS[ib]
        hsl = slice(DK * h, DK * (h + 1))
        for jb, (j0, jl) in enumerate(IBLOCKS):
            tp = ps_ty.tile([128, 128], BF16, tag="ty", name="ty")
            nc.tensor.matmul(tp[:jl, :il], p[:il, j0 : j0 + jl],
                             ident[:il, :il], is_transpose=True,
                             skip_group_check=True)
            pT = sm.tile([128, 128], BF16, tag="pT", name="pT")
            nc.scalar.copy(pT[:jl, :il], tp[:jl, :il])
            nc.tensor.matmul(xT_ps[hsl, :il], vb[(b, jb)][:jl, hsl],
                             pT[:jl, :il],
                             start=(jb == 0), stop=(jb == 1),
                             skip_group_check=True,
                             tile_position=(0, DK * h))

    def emit_proj(b, ib, xT_ps):
        i0, il = IBLOCKS[ib]
        xT_sb = sm.tile([128, 128], BF16, tag="xT_sb", name="xT_sb")
        nc.scalar.copy(xT_sb[:, :il], xT_ps[:, :il])
        y_ps = ps_ty.tile([128, 128], F32, tag="ty", name="ty")
        nc.tensor.matmul(y_ps[:il, :], xT_sb[:, :il], wo,
                         skip_group_check=True)
        y_sb = keep.tile([128, 128], F32, tag=f"y_out{b}_{ib}",
                         name=f"y_out{b}_{ib}")
        nc.vector.tensor_add(y_sb[:il, :], y_ps[:il, :],
                             reph["bo_b"][:il, :])
        nc.scalar.dma_start(out[b, i0 : i0 + il, :], y_sb[:il, :])

    def new_s2(hr):
        # full-bank rows (512 f32 = 2048B) so per-row matmul writes land
        # bank-aligned; only [:, :T] is used
        return ps_s2.tile([128, 512], F32, tag=f"s2h{hr}", name=f"s2h{hr}")

    # ---------------- main schedule ----------------
    # Block (0,0): interleave minimal prep with the first streams so the
    # PE never sits behind the full prep dependency chain.
    emit_rel_qw(0)
    s2ps = {0: new_s2(0), 1: new_s2(1)}
    emit_stream(0, 0, 0, s2ps[0], [(0, 32)])       # small first chunk
    emit_rel_qw(1)
    emit_stream(0, 0, 0, s2ps[0], [(32, 32), (64, 64)])
    emit_stream(0, 0, 1, s2ps[1], [(0, 64), (64, 64)])
    emit_rest_prep()
    emit_content(0, 0, 0, s2ps[0])
    emit_content(0, 0, 1, s2ps[1])
    xT_ps = ps_x.tile([128, 128], F32, tag="xT", name="xT")
    for h in (2, 3, 0, 1):
        p = emit_head_scores(0, 0, h, s2ps)
        emit_head_pv(0, 0, h, p, xT_ps)
    emit_proj(0, 0, xT_ps)

    # middle blocks
    for (b, ib) in ((0, 1), (1, 0)):
        s2ps = {0: new_s2(0), 1: new_s2(1)}
        for hr in range(H_REL):
            emit_stream(b, ib, hr, s2ps[hr], chunks_for(IBLOCKS[ib][1]))
            emit_content(b, ib, hr, s2ps[hr])
        xT_ps = ps_x.tile([128, 128], F32, tag="xT", name="xT")
        for h in (2, 3, 0, 1):
            p = emit_head_scores(b, ib, h, s2ps)
            emit_head_pv(b, ib, h, p, xT_ps)
        emit_proj(b, ib, xT_ps)

    # last block: abs heads fully before the rel streams; h2's softmax
    # hidden under the hr=1 stream; only h3's chain + proj remain at the end
    b, ib = BPC - 1, 1
    xT_ps = ps_x.tile([128, 128], F32, tag="xT", name="xT")
    for h in (0, 1):
        p = emit_head_scores(b, ib, h, None)
        emit_head_pv(b, ib, h, p, xT_ps)
    s2ps = {0: new_s2(0), 1: new_s2(1)}
    emit_stream(b, ib, 0, s2ps[0], chunks_for(IBLOCKS[ib][1]))
    emit_content(b, ib, 0, s2ps[0])
    p2 = emit_head_scores(b, ib, 2, s2ps)
    emit_stream(b, ib, 1, s2ps[1], chunks_for(IBLOCKS[ib][1]))
    emit_content(b, ib, 1, s2ps[1])
    emit_head_pv(b, ib, 2, p2, xT_ps)
    p3 = emit_head_scores(b, ib, 3, s2ps)
    emit_head_pv(b, ib, 3, p3, xT_ps)
    emit_proj(b, ib, xT_ps)


def build_nc():
    nc = bacc.Bacc(trn_type="TRN2")
    io = {}
    io["inpack"] = nc.dram_tensor(
        "inpack", [128, IN_COLS], BF16, kind="ExternalInput").ap()
    # fp8 bytes carried as bf16: [h, b, d, i, j/2]
    io["rel_kernel"] = nc.dram_tensor(
        "rel_kernel", [H_REL, BPC, D, T, T // 2], BF16, kind="ExternalInput"
    ).ap()
    io["out"] = nc.dram_tensor("out", [BPC, T, D], F32,
                               kind="ExternalOutput").ap()

    with tile.TileContext(nc) as tc:
        with ExitStack() as ctx:
            build_kernel(ctx, tc, io)
    nc.compile()
    return nc


_NC_CACHE = None


def _get_nc():
    global _NC_CACHE
    if _NC_CACHE is None:
        _NC_CACHE = build_nc()
    return _NC_CACHE


def make_in_maps(inputs):
    """Shard full inputs into per-core input maps (layout/dtype work only)."""
    import ml_dtypes
    bf = ml_dtypes.bfloat16
    f32 = np.float32
    g = {k: np.asarray(inputs[k], dtype=f32) for k in
         ["Wq", "bq", "Wk", "bk", "Wv", "bv", "abs_q_w", "abs_q_b",
          "abs_k_w", "abs_k_b", "rel_k_w", "rel_k_b", "rel_bias",
          "Wo", "bo"]}
    wp = np.zeros((128, WPACK_COLS), f32)

    def put(nm, arr):
        o = WPACK_OFF[nm]
        arr = np.asarray(arr, f32)
        if arr.ndim == 1:
            arr = arr[:, None]
        wp[: arr.shape[0], o : o + arr.shape[1]] = arr

    put("Wq", g["Wq"]); put("Wk", g["Wk"]); put("Wv", g["Wv"])
    put("Wo", g["Wo"]); put("bq", g["bq"]); put("bk", g["bk"])
    for hr in range(H_REL):
        put(f"rkwT{hr}", g["rel_k_w"][hr].T)  # [32 o, 128 d]
        put(f"rkb{hr}", g["rel_k_b"][hr])
        put(f"rbias{hr}", g["rel_bias"][0, hr, 0, :])
        put(f"bqrb{hr}", g["bq"][DK * (H_ABS + hr) : DK * (H_ABS + hr + 1)]
            + g["rel_bias"][0, hr, 0, :])
    for hh in range(H_ABS):
        put(f"aqw{hh}", g["abs_q_w"][hh])
        put(f"akw{hh}", g["abs_k_w"][hh])
        put(f"aqb{hh}", g["abs_q_b"][hh])
        put(f"akb{hh}", g["abs_k_b"][hh])
    put("bvb", np.tile(g["bv"][None, :], (128, 1)))
    put("bob", np.tile(g["bo"][None, :], (128, 1)))

    query = np.asarray(inputs["query"], dtype=f32)
    key = np.asarray(inputs["key"], dtype=f32)
    value = np.asarray(inputs["value"], dtype=f32)
    mask_i = np.asarray(inputs["mask"], dtype=np.int32)[:, 0]  # [B, T, T]
    absk = np.asarray(inputs["abs_kernel"], dtype=f32)

    # rel_kernel: fp8 e4m3, host-transposed to [h, B, d, i, j]
    rk8 = np.asarray(inputs["rel_kernel"], dtype=f32).astype(
        ml_dtypes.float8_e4m3fn)                     # [h, B, i, j, d]
    rkT = np.ascontiguousarray(rk8.transpose(0, 1, 4, 2, 3))  # [h,B,d,i,j]
    relk = rkT.view(np.uint16).view(bf)              # [h, B, d, i, j/2]

    in_maps = []
    for c in range(N_CORES):
        bs = slice(c * BPC, (c + 1) * BPC)
        ip = np.zeros((128, IN_COLS), f32)
        ip[:, :WPACK_COLS] = wp
        ip[:, IN_OFF_ID : IN_OFF_ID + 128] = np.eye(128, dtype=f32)
        ip[:, IN_OFF_Q : IN_OFF_Q + TT] = query[bs].reshape(TT, 128).T
        ip[:, IN_OFF_K : IN_OFF_K + TT] = key[bs].reshape(TT, 128).T
        ip[:, IN_OFF_V : IN_OFF_V + TT] = value[bs].reshape(TT, 128).T
        ip[:, IN_OFF_A0 : IN_OFF_A0 + TT] = absk[0, bs].reshape(TT, 128).T
        ip[:, IN_OFF_A1 : IN_OFF_A1 + TT] = absk[1, bs].reshape(TT, 128).T
        for bl in range(BPC):
            for ib, (i0, il) in enumerate(IBLOCKS):
                o = IN_OFF_M + (bl * 2 + ib) * T
                ip[:il, o : o + T] = mask_i[c * BPC + bl, i0 : i0 + il, :]
        m = {
            "inpack": np.ascontiguousarray(ip.astype(bf)),
            "rel_kernel": np.ascontiguousarray(relk[:, bs]),
        }
        in_maps.append(m)
    return in_maps


def kernel(**inputs) -> np.ndarray:
    nc = _get_nc()
    in_maps = make_in_maps(inputs)
    res = run_bass_kernel_spmd(nc, in_maps, core_ids=list(range(N_CORES)))
    return np.concatenate([r["out"] for r in res.results], axis=0)


if __name__ == "__main__":
    nc = build_nc()
    print("built ok")
